# revision 1
# baseline (speedup 1.0000x reference)
"""Trainium2 Bass kernel for nn_MultiHeadAttention_73589969649754
(gnn_message_passing / graph cross-attention).

Strategy:
  - Edges sorted by destination node (host-side index prep); each of the
    8 cores owns a contiguous node range per side, split at node
    boundaries so no segment straddles cores.
  - Per core: the 4 input projections are computed as one fused GEMM
    per side (node @ [Wk;Wv].T) over a 2500-node shard, then AllGathered
    (bf16) so every core holds the full [20480, 1024] K|V tables.
  - Edge phase: destination-sorted edges are packed into windows of
    <=128 consecutive nodes / <=1024 edge slots.  Per 128-edge block:
    dma_gather pulls K|V rows, DVE+ACT compute the edge dot product and
    exp (softmax without max-subtraction - safe at these magnitudes),
    and a one-hot matmul on the PE performs the weighted segment-sum
    directly in transposed [channel, node] orientation.  The output
    GEMM (Wo) consumes that orientation with no transposes; the softmax
    denominator z is recovered in the same PSUM pass and divided after
    the GEMM (column scaling commutes).  LeakyReLU+bias on DVE.
  - Host reassembles per-core [512, W*128] outputs by column map.
"""

import math

import numpy as np

N = 20000
E = 160000
C = 512
NCORES = 8
TEMP = float(np.sqrt(C))
NEG = 0.01
NPC = N // NCORES            # 2500 nodes per GEMM shard
NTILES = math.ceil(NPC / 128)
NPAD = NTILES * 128          # 2560 padded shard rows
BLK = 128                    # edges per block
BPW = 8                      # blocks per window
WCAP = BPW * BLK             # 1024 edge slots per window
DUMMY_REL = 999.0
SKIP_AG = False
SKIP_C = False


def _table_row(n):
    return (n // NPC) * NPAD + (n % NPC)


def _prep_side(seg_dst, seg_src):
    seg_dst = np.asarray(seg_dst, np.int64)
    seg_src = np.asarray(seg_src, np.int64)
    perm = np.argsort(seg_dst, kind="stable")
    sd = seg_dst[perm]
    ss = seg_src[perm]

    node_b = [0]
    for c in range(1, NCORES):
        node_b.append(int(sd[min(c * E // NCORES, E - 1)]))
    node_b.append(N)
    for i in range(1, len(node_b)):
        node_b[i] = max(node_b[i], node_b[i - 1])
    edge_b = [int(np.searchsorted(sd, nb, "left")) for nb in node_b]

    deg = np.bincount(sd, minlength=N)

    cores = []
    max_w = 0
    for c in range(NCORES):
        n0, n1 = node_b[c], node_b[c + 1]
        e0 = edge_b[c]
        wins = []
        n, e = n0, e0
        while n < n1:
            wn = we = 0
            while n + wn < n1 and wn < BLK and we + deg[n + wn] <= WCAP:
                we += deg[n + wn]
                wn += 1
            assert wn > 0, "node degree exceeds window capacity"
            wins.append((n, wn, e, we))
            n += wn
            e += we
        assert e == edge_b[c + 1]
        cores.append((n0, n1, wins, sd, ss))
        max_w = max(max_w, len(wins))
    return cores, max_w


def _wrap_idx16(idx_flat):
    """[n] -> [128, n//16] int16, i at [i%16, i//16], replicated x8."""
    n = idx_flat.shape[0]
    a = idx_flat.reshape(n // 16, 16).T.astype(np.int16)
    return np.ascontiguousarray(np.tile(a, (8, 1)))


def _build_core_arrays(cores, W):
    out = []
    for (n0, n1, wins, sd, ss) in cores:
        srcrow = np.zeros((W, WCAP), np.int64)
        dstrow = np.zeros((W, WCAP), np.int64)
        dstrel = np.full((W, WCAP), DUMMY_REL, np.float32)
        colnode = np.full(W * BLK, -1, np.int64)
        for w, (fn, wn, es, ne) in enumerate(wins):
            srcrow[w, :ne] = _table_row(ss[es:es + ne])
            dstrow[w, :ne] = _table_row(sd[es:es + ne])
            dstrel[w, :ne] = (sd[es:es + ne] - fn).astype(np.float32)
            colnode[w * BLK: w * BLK + wn] = np.arange(fn, fn + wn)
        # wrapped int16 index tiles: [128, W*64]
        sidx = np.concatenate(
            [_wrap_idx16(srcrow[w]) for w in range(W)], axis=1)
        didx = np.concatenate(
            [_wrap_idx16(dstrow[w]) for w in range(W)], axis=1)
        # dstrel as [128, W*8]: [p, w*8+b] = rel of edge b*128+p in window w
        drel = np.ascontiguousarray(
            dstrel.reshape(W, BPW, BLK).transpose(2, 0, 1).reshape(BLK, W * BPW))
        out.append(dict(sidx=sidx, didx=didx, drel=drel, colnode=colnode))
    return out


def _build_program(W):
    import concourse.bacc as bacc
    import concourse.tile as tile
    from concourse import mybir

    dt = mybir.dt
    f32, bf16, i16 = dt.float32, dt.bfloat16, dt.int16
    AF = mybir.ActivationFunctionType
    OP = mybir.AluOpType

    nc = bacc.Bacc("TRN2", target_bir_lowering=False, debug=False,
                   enable_asserts=True, num_devices=NCORES)

    # ---- I/O ----
    nT_in = {s: nc.dram_tensor(f"nT_{s}", [C, NPAD], bf16,
                               kind="ExternalInput").ap() for s in "LR"}
    wkvT = nc.dram_tensor("wkvT", [128, 4 * 1024], bf16,
                          kind="ExternalInput").ap()
    woT = nc.dram_tensor("woT", [128, 4 * 512], bf16,
                         kind="ExternalInput").ap()
    bo_in = nc.dram_tensor("bo", [128, 4], f32, kind="ExternalInput").ap()
    iota_in = nc.dram_tensor("iota", [128, 128], f32,
                             kind="ExternalInput").ap()
    sidx_in = {s: nc.dram_tensor(f"sidx_{s}", [128, W * 64], i16,
                                 kind="ExternalInput").ap() for s in "LR"}
    didx_in = {s: nc.dram_tensor(f"didx_{s}", [128, W * 64], i16,
                                 kind="ExternalInput").ap() for s in "LR"}
    drel_in = {s: nc.dram_tensor(f"drel_{s}", [128, W * BPW], f32,
                                 kind="ExternalInput").ap() for s in "LR"}
    hT_out = {s: nc.dram_tensor(f"hT_{s}", [C, W * BLK], f32,
                                kind="ExternalOutput").ap() for s in "LR"}

    # ---- internal DRAM ----
    tkv_sh = {s: nc.dram_tensor(f"tkv_sh_{s}", [NPAD, 2 * C], bf16).ap()
              for s in "LR"}
    shared = "Shared" if NCORES > 4 else "Local"
    tkv = {s: nc.dram_tensor(f"tkv_{s}", [NCORES * NPAD, 2 * C], bf16,
                             addr_space=shared).ap() for s in "LR"}

    with tile.TileContext(nc) as tc:
        with tc.tile_pool(name="const", bufs=1) as cpool:
            # constants
            wkvT_sb = cpool.tile([128, 4 * 1024], bf16)
            nc.sync.dma_start(wkvT_sb[:], wkvT[:, :])
            woT_sb = cpool.tile([128, 4 * 512], bf16)
            nc.sync.dma_start(woT_sb[:], woT[:, :])
            bo_sb = cpool.tile([128, 4], f32)
            nc.sync.dma_start(bo_sb[:], bo_in[:, :])
            iota_sb = cpool.tile([128, 128], f32)
            nc.sync.dma_start(iota_sb[:], iota_in[:, :])
            ones_col = cpool.tile([128, 1], bf16)
            nc.vector.memset(ones_col[:], 1.0)
            ones_row = cpool.tile([1, 128], bf16)
            nc.vector.memset(ones_row[:], 1.0)
            idx_sb = {}
            for s in "LR":
                sidx_sb = cpool.tile([128, W * 64], i16, tag=f"sidx{s}")
                nc.sync.dma_start(sidx_sb[:], sidx_in[s][:, :])
                didx_sb = cpool.tile([128, W * 64], i16, tag=f"didx{s}")
                nc.sync.dma_start(didx_sb[:], didx_in[s][:, :])
                drel_sb = cpool.tile([128, W * BPW], f32, tag=f"drel{s}")
                nc.sync.dma_start(drel_sb[:], drel_in[s][:, :])
                idx_sb[s] = (sidx_sb, didx_sb, drel_sb)

            # ---- phase A: projection GEMMs into table shards ----
            with (
                tc.tile_pool(name="feat", bufs=1) as fpool,
                tc.tile_pool(name="gemm_sb", bufs=3) as gsb,
                tc.tile_pool(name="psum_gemm", bufs=2, space="PSUM") as pg,
            ):
                for s in "LR":
                    feat = []
                    for cc in range(4):
                        t = fpool.tile([128, NPAD], bf16, tag=f"feat{s}{cc}")
                        nc.sync.dma_start(
                            t[:], nT_in[s][cc * 128:(cc + 1) * 128, :])
                        feat.append(t)
                    for ti in range(NTILES):
                        sb = gsb.tile([128, 1024], bf16)
                        for half in range(2):
                            ps = pg.tile([128, 512], f32)
                            for cc in range(4):
                                nc.tensor.matmul(
                                    ps[:],
                                    lhsT=feat[cc][:, ti * 128:(ti + 1) * 128],
                                    rhs=wkvT_sb[:, cc * 1024 + half * 512:
                                                cc * 1024 + half * 512 + 512],
                                    start=(cc == 0), stop=(cc == 3))
                            nc.scalar.copy(
                                sb[:, half * 512:(half + 1) * 512], ps[:])
                        nc.sync.dma_start(
                            tkv_sh[s][ti * 128:(ti + 1) * 128, :], sb[:])

                # ---- phase B: AllGather both tables ----
                if not SKIP_AG:
                    for s in "LR":
                        nc.gpsimd.collective_compute(
                            "AllGather", mybir.AluOpType.bypass,
                            replica_groups=[list(range(NCORES))],
                            ins=[tkv_sh[s]], outs=[tkv[s]])

            # ---- phase C: edge processing ----
            with (
                tc.tile_pool(name="gath", bufs=3) as gpool,
                tc.tile_pool(name="blk", bufs=4) as sp,
                tc.tile_pool(name="ohs", bufs=2 * BPW) as ohpool,
                tc.tile_pool(name="tail", bufs=3) as tp,
                tc.tile_pool(name="pmsg", bufs=2, space="PSUM") as pmsg,
                tc.tile_pool(name="pz", bufs=2, space="PSUM") as pz,
                tc.tile_pool(name="pzbc", bufs=2, space="PSUM") as pzbc,
                tc.tile_pool(name="ph", bufs=2, space="PSUM") as ph,
            ):
                nidx_reg = nc.gpsimd.to_reg(WCAP)
                for s, o in ((() if SKIP_C else (("L", "R"), ("R", "L")))):
                    sidx_sb, didx_sb, drel_sb = idx_sb[s]
                    hacc = cpool.tile([128, 4 * W * 128], f32, tag=f"hacc{s}")
                    for w in range(W):
                        kv = gpool.tile([128, BPW, 2 * C], bf16, tag="kv")
                        nc.gpsimd.dma_gather(
                            kv[:], tkv[o][:, :], sidx_sb[:, w * 64:(w + 1) * 64],
                            WCAP, nidx_reg, 2 * C)
                        kd = gpool.tile([128, BPW, C], bf16, tag="kd")
                        nc.gpsimd.dma_gather(
                            kd[:], tkv[s][:, 0:C], didx_sb[:, w * 64:(w + 1) * 64],
                            WCAP, nidx_reg, C, elem_step=2 * C)

                        msgT_ps = pmsg.tile([128, 512], f32)
                        z_ps = pz.tile([1, 128], f32)
                        sacc = sp.tile([128, BPW], f32, tag="sacc")
                        for b in range(BPW):
                            prod = sp.tile([128, C], bf16, tag="prod")
                            nc.vector.tensor_tensor(
                                prod[:], kv[:, b, 0:C], kd[:, b, :], op=OP.mult)
                            nc.scalar.activation(
                                prod[:], prod[:], AF.Copy, bias=0.0,
                                scale=1.0, accum_out=sacc[:, b:b + 1])
                        eh = sp.tile([128, BPW], f32, tag="eh")
                        nc.scalar.activation(eh[:], sacc[:], AF.Exp,
                                             scale=1.0 / TEMP)
                        ohs = []
                        for b in range(BPW):
                            oh = ohpool.tile([128, 128], bf16, tag="oh")
                            nc.vector.tensor_scalar(
                                oh[:], iota_sb[:],
                                drel_sb[:, w * BPW + b: w * BPW + b + 1],
                                eh[:, b:b + 1], op0=OP.is_equal, op1=OP.mult)
                            ohs.append(oh)
                        for cc in range(4):
                            for b in range(BPW):
                                nc.tensor.matmul(
                                    msgT_ps[:, cc * 128:(cc + 1) * 128],
                                    lhsT=kv[:, b, C + cc * 128: C + (cc + 1) * 128],
                                    rhs=ohs[b][:],
                                    start=(b == 0), stop=(b == BPW - 1))
                        for b in range(BPW):
                            nc.tensor.matmul(
                                z_ps[:], lhsT=ones_col[:], rhs=ohs[b][:],
                                start=(b == 0), stop=(b == BPW - 1))

                        # window tail
                        zm = tp.tile([1, 128], f32, tag="zm")
                        nc.vector.tensor_scalar_max(zm[:], z_ps[:], 1e-30)
                        zr = tp.tile([1, 128], f32, tag="zr")
                        nc.vector.reciprocal(zr[:], zm[:])
                        zrb = tp.tile([1, 128], bf16, tag="zrb")
                        nc.vector.tensor_copy(zrb[:], zr[:])
                        zbc_ps = pzbc.tile([128, 128], f32)
                        nc.tensor.matmul(zbc_ps[:], lhsT=ones_row[:], rhs=zrb[:],
                                         start=True, stop=True)
                        zbc = tp.tile([128, 128], f32, tag="zbc")
                        nc.vector.tensor_copy(zbc[:], zbc_ps[:])
                        msgT_sb = tp.tile([128, 512], bf16, tag="msgT")
                        for nch in range(4):
                            nc.vector.tensor_tensor(
                                msgT_sb[:, nch * 128:(nch + 1) * 128],
                                msgT_ps[:, nch * 128:(nch + 1) * 128],
                                zbc[:], op=OP.mult)
                        hT_ps = ph.tile([128, 512], f32)
                        for oc in range(4):
                            for cc in range(4):
                                nc.tensor.matmul(
                                    hT_ps[:, oc * 128:(oc + 1) * 128],
                                    lhsT=woT_sb[:, cc * 512 + oc * 128:
                                                cc * 512 + oc * 128 + 128],
                                    rhs=msgT_sb[:, cc * 128:(cc + 1) * 128],
                                    start=(cc == 0), stop=(cc == 3))
                        for oc in range(4):
                            x = hacc[:, (oc * W + w) * 128:
                                     (oc * W + w) * 128 + 128]
                            nc.scalar.activation(
                                x, hT_ps[:, oc * 128:(oc + 1) * 128],
                                AF.Identity, bias=bo_sb[:, oc:oc + 1])
                            x2 = tp.tile([128, 128], f32, tag="x2")
                            nc.vector.tensor_scalar_mul(x2[:], x, NEG)
                            nc.vector.tensor_tensor(x, x, x2[:], op=OP.max)
                    for oc in range(4):
                        nc.sync.dma_start(
                            hT_out[s][oc * 128:(oc + 1) * 128, :],
                            hacc[:, oc * W * 128:(oc + 1) * W * 128])
    nc.compile()
    return nc


def _host_inputs(inputs):
    import ml_dtypes
    bf16 = ml_dtypes.bfloat16

    nl = np.asarray(inputs["node_left"], np.float32)
    nr = np.asarray(inputs["node_right"], np.float32)
    Wk = np.asarray(inputs["Wk"], np.float32)
    Wv = np.asarray(inputs["Wv"], np.float32)
    Wo = np.asarray(inputs["Wo"], np.float32)
    bo = np.asarray(inputs["bo"], np.float32)
    sl = np.asarray(inputs["segmentation_index_left"], np.int64)
    sr = np.asarray(inputs["segmentation_index_right"], np.int64)

    coresL, wL = _prep_side(sl, sr)
    coresR, wR = _prep_side(sr, sl)
    W = max(wL, wR)
    arrL = _build_core_arrays(coresL, W)
    arrR = _build_core_arrays(coresR, W)

    Wkv = np.concatenate([Wk, Wv], 0)               # [1024, 512]
    WkvT = Wkv.T                                    # [512, 1024]
    wkvT_arr = np.zeros((128, 4 * 1024), np.float32)
    for cc in range(4):
        wkvT_arr[:, cc * 1024:(cc + 1) * 1024] = \
            WkvT[cc * 128:(cc + 1) * 128, :]
    woT_arr = np.zeros((128, 4 * 512), np.float32)
    for cc in range(4):
        for oc in range(4):
            woT_arr[:, cc * 512 + oc * 128: cc * 512 + (oc + 1) * 128] = \
                Wo[oc * 128:(oc + 1) * 128, cc * 128:(cc + 1) * 128].T
    bo_arr = bo.reshape(4, 128).T.copy()            # [128, 4]
    iota_arr = np.broadcast_to(
        np.arange(128, dtype=np.float32)[None, :], (128, 128)).copy()

    def shardT(feat, c):
        sh = np.zeros((C, NPAD), np.float32)
        sh[:, :NPC] = feat[c * NPC:(c + 1) * NPC].T
        return np.ascontiguousarray(sh).astype(bf16)

    in_maps = []
    for c in range(NCORES):
        in_maps.append({
            "nT_L": shardT(nl, c),
            "nT_R": shardT(nr, c),
            "wkvT": wkvT_arr.astype(bf16),
            "woT": woT_arr.astype(bf16),
            "bo": bo_arr,
            "iota": iota_arr,
            "sidx_L": arrL[c]["sidx"],
            "didx_L": arrL[c]["didx"],
            "drel_L": arrL[c]["drel"],
            "sidx_R": arrR[c]["sidx"],
            "didx_R": arrR[c]["didx"],
            "drel_R": arrR[c]["drel"],
        })
    return in_maps, arrL, arrR, W


def _assemble(results, arrs, key):
    out = np.zeros((N, C), np.float32)
    for c in range(NCORES):
        hT = np.asarray(results[c][key], np.float32)
        cn = arrs[c]["colnode"]
        m = cn >= 0
        out[cn[m]] = hT[:, m].T
    return out


_RUN_KWARGS = {}


def kernel(**inputs):
    from concourse.bass_utils import run_bass_kernel_spmd

    in_maps, arrL, arrR, W = _host_inputs(inputs)
    nc = _build_program(W)
    res = run_bass_kernel_spmd(nc, in_maps, core_ids=list(range(NCORES)),
                               **_RUN_KWARGS)
    out_l = _assemble(res.results, arrL, "hT_L")
    out_r = _assemble(res.results, arrR, "hT_R")
    kernel.last_results = res
    kernel.last_nc = nc
    kernel.last_W = W
    return (out_l, out_r)



# revision 11
# speedup vs baseline: 1.1523x; 1.1523x over previous
"""Trainium2 Bass kernel for nn_MultiHeadAttention_73589969649754
(gnn_message_passing / graph cross-attention).

v2 strategy (score sharing):
  - Edges sorted by destination node per side; core c owns the node range
    [c*2500, (c+1)*2500) on both sides (node-aligned so the dst-K table is
    local).  Windows of <=128 consecutive nodes / <=1024 edge slots.
  - Phase A: per-core projection GEMMs.  R side emits a fused K|V table
    (AllGather #1, 5.24MB/rank); L side emits a local K table (no
    collective) and a V table (AllGather #2, 2.62MB/rank).
  - L pass: per window, gather (Kr|Vr)[src] rows (2KB/edge) from the AG'd
    R table and Kl[dst] rows (1KB/edge) from the LOCAL L K-table (these
    gathers overlap AllGather #1).  The edge dot product runs as a single
    DVE tensor_tensor_reduce per 128-edge block (no ACT accumulate), then
    eh = exp(S/temp) is computed once and written to a score shard
    (AllGather #3, 80KB/rank).  One-hot matmuls do the segment-sum in
    [channel, node] orientation; z recovered via ones-matmul; Wo GEMM,
    bias+LeakyReLU.
  - R pass: gathers only Vl[src] rows (1KB/edge) plus 256B score-table
    rows; a host-shipped one-hot mask selects each edge's eh (shared with
    the L pass - softmax numerators are identical on both sides).  No K
    gather, no dot product, no exp on the R side.
  - Outputs stored bf16 in [channel, window-slot] layout; host reassembles.
"""

import math

import numpy as np

N = 20000
E = 160000
C = 512
NCORES = 8
TEMP = float(np.sqrt(C))
NEG = 0.01
NPC = N // NCORES            # 2500 nodes per GEMM shard
NTILES = math.ceil(NPC / 128)
NPAD = NTILES * 128          # 2560 padded shard rows
BLK = 128                    # edges per block
BPW = 8                      # blocks per window
WCAP = BPW * BLK             # 1024 edge slots per window
DUMMY_REL = 999.0
SKIP_AG = False
SKIP_C = False
SKIP_L = False       # debug: skip the L window loop
ACC_ON_ACT = 6       # blocks/window whose dot-accum runs on ACT (rest DVE)
SKIP_R = False       # debug: skip the R window loop

# AllGather wall-time charge (ns): measured-table upper bound is ~70us for
# a 5.24MB/rank 8-way intra-chip AllGather; scale by bytes with a 20us
# latency floor.  AG#1 tkv_R 5.24MB -> 70us, AG#2 tv_L 2.62MB -> 35us,
# AG#3 escore 80KB -> 20us (floor).
AG_CHARGE_NS = 70e3 + 35e3 + 20e3


def _table_row(n):
    return (n // NPC) * NPAD + (n % NPC)


def _prep_side(seg_dst):
    """Sort edges by dst; node-aligned core ranges; pack windows.

    Returns (perm, cores, max_w, Lloc) where Lloc maps sorted-edge ->
    (core, window, slot) and cores[c] = (wins, e0) with wins =
    [(first_node, n_nodes, e_start, n_edges), ...].
    """
    seg_dst = np.asarray(seg_dst, np.int64)
    perm = np.argsort(seg_dst, kind="stable")
    sd = seg_dst[perm]
    deg = np.bincount(sd, minlength=N)
    edge_b = [int(np.searchsorted(sd, c * NPC, "left"))
              for c in range(NCORES)] + [E]

    cores = []
    max_w = 0
    for c in range(NCORES):
        n0, n1 = c * NPC, (c + 1) * NPC
        e0 = edge_b[c]
        wins = []
        n, e = n0, e0
        while n < n1:
            wn = we = 0
            while n + wn < n1 and wn < BLK and we + deg[n + wn] <= WCAP:
                we += deg[n + wn]
                wn += 1
            assert wn > 0, "node degree exceeds window capacity"
            wins.append((n, wn, e, we))
            n += wn
            e += we
        assert e == edge_b[c + 1]
        cores.append((wins, e0))
        max_w = max(max_w, len(wins))
    return perm, sd, cores, max_w


def _wrap_idx16(idx_flat):
    """[n] -> [128, n//16] int16, i at [i%16, i//16], replicated x8."""
    n = idx_flat.shape[0]
    a = idx_flat.reshape(n // 16, 16).T.astype(np.int16)
    return np.ascontiguousarray(np.tile(a, (8, 1)))


def _edge_locs(perm, cores, W):
    """Per original edge id: (core, window, slot) on this side."""
    core_of = np.empty(E, np.int32)
    win_of = np.empty(E, np.int32)
    slot_of = np.empty(E, np.int32)
    for c, (wins, e0) in enumerate(cores):
        for w, (fn, wn, es, ne) in enumerate(wins):
            ids = perm[es:es + ne]
            core_of[ids] = c
            win_of[ids] = w
            slot_of[ids] = np.arange(ne)
    return core_of, win_of, slot_of


def _host_inputs(inputs):
    import ml_dtypes
    bf16 = ml_dtypes.bfloat16

    nl = np.asarray(inputs["node_left"], np.float32)
    nr = np.asarray(inputs["node_right"], np.float32)
    Wk = np.asarray(inputs["Wk"], np.float32)
    Wv = np.asarray(inputs["Wv"], np.float32)
    Wo = np.asarray(inputs["Wo"], np.float32)
    bo = np.asarray(inputs["bo"], np.float32)
    sl = np.asarray(inputs["segmentation_index_left"], np.int64)
    sr = np.asarray(inputs["segmentation_index_right"], np.int64)

    permL, sdL, coresL, wL = _prep_side(sl)
    permR, sdR, coresR, wR = _prep_side(sr)
    W = max(wL, wR)

    # L-side edge locations (for escore addressing by the R side)
    LcoreE, LwinE, LslotE = _edge_locs(permL, coresL, W)

    # ---- per-core L arrays ----
    ssL = sr[permL]          # src (right) node per L-sorted edge
    arrL = []
    for c, (wins, e0) in enumerate(coresL):
        sidx = np.zeros((W, WCAP), np.int64)      # src rows in R table
        didx = np.zeros((W, WCAP), np.int64)      # dst rows in local L K tab
        drel = np.full((W, WCAP), DUMMY_REL, np.float32)
        colnode = np.full(W * BLK, -1, np.int64)
        for w, (fn, wn, es, ne) in enumerate(wins):
            sidx[w, :ne] = _table_row(ssL[es:es + ne])
            didx[w, :ne] = sdL[es:es + ne] - c * NPC
            drel[w, :ne] = (sdL[es:es + ne] - fn).astype(np.float32)
            colnode[w * BLK: w * BLK + wn] = np.arange(fn, fn + wn)
        sidx16 = np.concatenate([_wrap_idx16(sidx[w]) for w in range(W)], 1)
        didx16 = np.concatenate([_wrap_idx16(didx[w]) for w in range(W)], 1)
        drelT = np.ascontiguousarray(
            drel.reshape(W, BPW, BLK).transpose(2, 0, 1).reshape(BLK, W * BPW))
        arrL.append(dict(sidx=sidx16, didx=didx16, drel=drelT,
                         colnode=colnode))

    # ---- per-core R arrays ----
    ssR = sl[permR]          # src (left) node per R-sorted edge
    arrR = []
    for c, (wins, e0) in enumerate(coresR):
        vidx = np.zeros((W, WCAP), np.int64)      # src rows in L V table
        eidx = np.zeros((W, WCAP), np.int64)      # escore row (64-col rows)
        ecol = np.zeros((W, WCAP), np.int64)      # escore col within row
        emask_valid = np.zeros((W, WCAP), bool)
        drel = np.full((W, WCAP), DUMMY_REL, np.float32)
        colnode = np.full(W * BLK, -1, np.int64)
        for w, (fn, wn, es, ne) in enumerate(wins):
            ids = permR[es:es + ne]
            vidx[w, :ne] = _table_row(ssR[es:es + ne])
            # flat L position: core*(W*1024) + win*1024 + slot_p*8 + slot_b
            pL = LslotE[ids] % BLK
            bL = LslotE[ids] // BLK
            gpos = (LcoreE[ids].astype(np.int64) * W + LwinE[ids]) * WCAP \
                + pL * BPW + bL
            eidx[w, :ne] = gpos // 64
            ecol[w, :ne] = gpos % 64
            emask_valid[w, :ne] = True
            drel[w, :ne] = (sdR[es:es + ne] - fn).astype(np.float32)
            colnode[w * BLK: w * BLK + wn] = np.arange(fn, fn + wn)
        vidx16 = np.concatenate([_wrap_idx16(vidx[w]) for w in range(W)], 1)
        eidx16 = np.concatenate([_wrap_idx16(eidx[w]) for w in range(W)], 1)
        drelT = np.ascontiguousarray(
            drel.reshape(W, BPW, BLK).transpose(2, 0, 1).reshape(BLK, W * BPW))
        # emask [128, W*8*64]: [p, ((w*8)+b)*64 + col] = 1 for valid slots
        emask = np.zeros((BLK, W * BPW * 64), np.float32)
        wv, iv = np.nonzero(emask_valid)
        pv, bv = iv % BLK, iv // BLK
        emask[pv, (wv * BPW + bv) * 64 + ecol[wv, iv]] = 1.0
        arrR.append(dict(vidx=vidx16, eidx=eidx16, drel=drelT,
                         emask=emask.astype(bf16), colnode=colnode))

    # ---- shared constants ----
    Wkv = np.concatenate([Wk, Wv], 0)               # [1024, 512]
    WkvT = Wkv.T                                    # [512, 1024]
    wkvT_arr = np.zeros((128, 4 * 1024), np.float32)
    for cc in range(4):
        wkvT_arr[:, cc * 1024:(cc + 1) * 1024] = \
            WkvT[cc * 128:(cc + 1) * 128, :]
    woT_arr = np.zeros((128, 4 * 512), np.float32)
    for cc in range(4):
        for oc in range(4):
            woT_arr[:, cc * 512 + oc * 128: cc * 512 + (oc + 1) * 128] = \
                Wo[oc * 128:(oc + 1) * 128, cc * 128:(cc + 1) * 128].T
    bo_arr = bo.reshape(4, 128).T.copy()            # [128, 4]
    iota_arr = np.broadcast_to(
        np.arange(128, dtype=np.float32)[None, :], (128, 128))

    def shardT(feat, c):
        sh = np.zeros((C, NPAD), np.float32)
        sh[:, :NPC] = feat[c * NPC:(c + 1) * NPC].T
        return np.ascontiguousarray(sh).astype(bf16)

    in_maps = []
    for c in range(NCORES):
        in_maps.append({
            "nT_L": shardT(nl, c),
            "nT_R": shardT(nr, c),
            "wkvT": wkvT_arr.astype(bf16),
            "woT": woT_arr.astype(bf16),
            "bo": bo_arr,
            "iota": np.ascontiguousarray(iota_arr).astype(bf16),
            "sidx_L": arrL[c]["sidx"],
            "didx_L": arrL[c]["didx"],
            "drel_L": arrL[c]["drel"],
            "vidx_R": arrR[c]["vidx"],
            "eidx_R": arrR[c]["eidx"],
            "drel_R": arrR[c]["drel"],
            "emask_R": arrR[c]["emask"],
        })
    return in_maps, arrL, arrR, W


def _build_program(W):
    import concourse.bacc as bacc
    import concourse.tile as tile
    from concourse import mybir

    dt = mybir.dt
    f32, bf16, i16 = dt.float32, dt.bfloat16, dt.int16
    AF = mybir.ActivationFunctionType
    OP = mybir.AluOpType

    nc = bacc.Bacc("TRN2", target_bir_lowering=False, debug=False,
                   enable_asserts=True, num_devices=NCORES)

    # ---- I/O ----
    nT_in = {s: nc.dram_tensor(f"nT_{s}", [C, NPAD], bf16,
                               kind="ExternalInput").ap() for s in "LR"}
    wkvT = nc.dram_tensor("wkvT", [128, 4 * 1024], bf16,
                          kind="ExternalInput").ap()
    woT = nc.dram_tensor("woT", [128, 4 * 512], bf16,
                         kind="ExternalInput").ap()
    bo_in = nc.dram_tensor("bo", [128, 4], f32, kind="ExternalInput").ap()
    iota_in = nc.dram_tensor("iota", [128, 128], bf16,
                             kind="ExternalInput").ap()
    sidx_in = nc.dram_tensor("sidx_L", [128, W * 64], i16,
                             kind="ExternalInput").ap()
    didx_in = nc.dram_tensor("didx_L", [128, W * 64], i16,
                             kind="ExternalInput").ap()
    drelL_in = nc.dram_tensor("drel_L", [128, W * BPW], f32,
                              kind="ExternalInput").ap()
    vidx_in = nc.dram_tensor("vidx_R", [128, W * 64], i16,
                             kind="ExternalInput").ap()
    eidx_in = nc.dram_tensor("eidx_R", [128, W * 64], i16,
                             kind="ExternalInput").ap()
    drelR_in = nc.dram_tensor("drel_R", [128, W * BPW], f32,
                              kind="ExternalInput").ap()
    emask_in = nc.dram_tensor("emask_R", [128, W * BPW * 64], bf16,
                              kind="ExternalInput").ap()
    hT_out = {s: nc.dram_tensor(f"hT_{s}", [C, W * BLK], bf16,
                                kind="ExternalOutput").ap() for s in "LR"}

    # ---- internal DRAM ----
    tkv_shR = nc.dram_tensor("tkv_shR", [NPAD, 2 * C], bf16).ap()
    tk_L = nc.dram_tensor("tk_L", [NPAD, C], bf16).ap()       # local only
    tv_shL = nc.dram_tensor("tv_shL", [NPAD, C], bf16).ap()
    esc_sh = nc.dram_tensor("esc_sh", [W, 128, BPW], f32).ap()
    tkv_R = nc.dram_tensor("tkv_R", [NCORES * NPAD, 2 * C], bf16,
                           addr_space="Shared").ap()
    tv_L = nc.dram_tensor("tv_L", [NCORES * NPAD, C], bf16,
                          addr_space="Shared").ap()
    esc_full = nc.dram_tensor("esc_full", [NCORES * W * 16, 64], f32,
                              addr_space="Shared").ap()

    with tile.TileContext(nc) as tc:
        with tc.tile_pool(name="const", bufs=1) as cpool:
            # constants
            wkvT_sb = cpool.tile([128, 4 * 1024], bf16)
            nc.sync.dma_start(wkvT_sb[:], wkvT[:, :])
            woT_sb = cpool.tile([128, 4 * 512], bf16)
            nc.sync.dma_start(woT_sb[:], woT[:, :])
            bo_sb = cpool.tile([128, 4], f32)
            nc.sync.dma_start(bo_sb[:], bo_in[:, :])
            iota_sb = cpool.tile([128, 128], bf16)
            nc.sync.dma_start(iota_sb[:], iota_in[:, :])
            ones_col = cpool.tile([128, 1], bf16)
            nc.vector.memset(ones_col[:], 1.0)
            ones_row = cpool.tile([1, 128], bf16)
            nc.vector.memset(ones_row[:], 1.0)
            sidx_sb = cpool.tile([128, W * 64], i16)
            nc.sync.dma_start(sidx_sb[:], sidx_in[:, :])
            didx_sb = cpool.tile([128, W * 64], i16)
            nc.sync.dma_start(didx_sb[:], didx_in[:, :])
            drelL_sb = cpool.tile([128, W * BPW], f32)
            nc.sync.dma_start(drelL_sb[:], drelL_in[:, :])
            vidx_sb = cpool.tile([128, W * 64], i16)
            nc.sync.dma_start(vidx_sb[:], vidx_in[:, :])
            eidx_sb = cpool.tile([128, W * 64], i16)
            nc.sync.dma_start(eidx_sb[:], eidx_in[:, :])
            drelR_sb = cpool.tile([128, W * BPW], f32)
            nc.sync.dma_start(drelR_sb[:], drelR_in[:, :])
            emask_sb = cpool.tile([128, W * BPW * 64], bf16)
            nc.sync.dma_start(emask_sb[:], emask_in[:, :])
            hacc = cpool.tile([128, 4 * W * 128], bf16)

            # ---- phase A: projection GEMMs ----
            with (
                tc.tile_pool(name="feat", bufs=1) as fpool,
                tc.tile_pool(name="gemm_sb", bufs=3) as gsb,
                tc.tile_pool(name="psum_gemm", bufs=2, space="PSUM") as pg,
            ):
                # R side first (feeds AG#1, the critical collective)
                for s in "RL":
                    feat = []
                    for cc in range(4):
                        t = fpool.tile([128, NPAD], bf16, tag=f"feat{s}{cc}")
                        nc.sync.dma_start(
                            t[:], nT_in[s][cc * 128:(cc + 1) * 128, :])
                        feat.append(t)
                    for ti in range(NTILES):
                        sb = gsb.tile([128, 1024], bf16)
                        for half in range(2):
                            ps = pg.tile([128, 512], f32)
                            for cc in range(4):
                                nc.tensor.matmul(
                                    ps[:],
                                    lhsT=feat[cc][:, ti * 128:(ti + 1) * 128],
                                    rhs=wkvT_sb[:, cc * 1024 + half * 512:
                                                cc * 1024 + half * 512 + 512],
                                    start=(cc == 0), stop=(cc == 3))
                            nc.vector.tensor_copy(
                                sb[:, half * 512:(half + 1) * 512], ps[:])
                        r0 = ti * 128
                        if s == "R":
                            nc.sync.dma_start(
                                tkv_shR[r0:r0 + 128, :], sb[:])
                        else:
                            nc.sync.dma_start(
                                tk_L[r0:r0 + 128, :], sb[:, 0:C])
                            nc.sync.dma_start(
                                tv_shL[r0:r0 + 128, :], sb[:, C:2 * C])
                    if s == "R" and not SKIP_AG:
                        nc.gpsimd.collective_compute(
                            "AllGather", mybir.AluOpType.bypass,
                            replica_groups=[list(range(NCORES))],
                            ins=[tkv_shR], outs=[tkv_R])
                if not SKIP_AG:
                    nc.gpsimd.collective_compute(
                        "AllGather", mybir.AluOpType.bypass,
                        replica_groups=[list(range(NCORES))],
                        ins=[tv_shL], outs=[tv_L])

            nidx_reg = nc.gpsimd.to_reg(WCAP)

            # ---- phase C-L: score + left messages ----
            with (
                tc.tile_pool(name="gath", bufs=2) as gpool,
                tc.tile_pool(name="kdp", bufs=2) as kdpool,
                tc.tile_pool(name="blk", bufs=4) as sp,
                tc.tile_pool(name="ohs", bufs=2 * BPW) as ohpool,
                tc.tile_pool(name="tail", bufs=3) as tp,
                tc.tile_pool(name="pmsg", bufs=2, space="PSUM") as pmsg,
                tc.tile_pool(name="pz", bufs=2, space="PSUM") as pz,
                tc.tile_pool(name="pzbc", bufs=2, space="PSUM") as pzbc,
                tc.tile_pool(name="ph", bufs=2, space="PSUM") as ph,
            ):
                for w in ([] if (SKIP_C or SKIP_L) else range(W)):
                    kv = gpool.tile([128, BPW, 2 * C], bf16, tag="kv")
                    nc.gpsimd.dma_gather(
                        kv[:], tkv_R[:, :], sidx_sb[:, w * 64:(w + 1) * 64],
                        WCAP, nidx_reg, 2 * C)
                    kd = kdpool.tile([128, BPW, C], bf16, tag="kd")
                    nc.gpsimd.dma_gather(
                        kd[:], tk_L[:, :], didx_sb[:, w * 64:(w + 1) * 64],
                        WCAP, nidx_reg, C)

                    # edge dot product: DVE multiply, accumulation split
                    # between ACT (copy+accum) and DVE (reduce) for balance
                    sacc = sp.tile([128, BPW], f32, tag="sacc")
                    for b in range(BPW):
                        scr = sp.tile([128, C], bf16, tag="scr")
                        nc.vector.tensor_tensor(
                            scr[:], kv[:, b, 0:C], kd[:, b, :], op=OP.mult)
                        if b < ACC_ON_ACT:
                            nc.scalar.activation(
                                scr[:], scr[:], AF.Copy, bias=0.0, scale=1.0,
                                accum_out=sacc[:, b:b + 1])
                        else:
                            nc.vector.tensor_reduce(
                                sacc[:, b:b + 1], scr[:],
                                axis=mybir.AxisListType.X, op=OP.add)
                    eh = sp.tile([128, BPW], f32, tag="eh")
                    nc.scalar.activation(eh[:], sacc[:], AF.Exp,
                                         scale=1.0 / TEMP)
                    # escore shard write: esc[w, p, b] = eh[p, b]
                    nc.sync.dma_start(esc_sh[w, :, :], eh[:])

                    ohs = []
                    for b in range(BPW):
                        oh = ohpool.tile([128, 128], bf16, tag="oh")
                        nc.vector.tensor_scalar(
                            oh[:], iota_sb[:],
                            drelL_sb[:, w * BPW + b: w * BPW + b + 1],
                            eh[:, b:b + 1], op0=OP.is_equal, op1=OP.mult)
                        ohs.append(oh)

                    msgT_ps = pmsg.tile([128, 512], f32)
                    z_ps = pz.tile([1, 128], f32)
                    for cc in range(4):
                        for b in range(BPW):
                            nc.tensor.matmul(
                                msgT_ps[:, cc * 128:(cc + 1) * 128],
                                lhsT=kv[:, b, C + cc * 128: C + (cc + 1) * 128],
                                rhs=ohs[b][:],
                                start=(b == 0), stop=(b == BPW - 1))
                    for b in range(BPW):
                        nc.tensor.matmul(
                            z_ps[:], lhsT=ones_col[:], rhs=ohs[b][:],
                            start=(b == 0), stop=(b == BPW - 1))

                    _window_tail(nc, tc, mybir, w, msgT_ps, z_ps,
                                 tp, pzbc, ph, ones_row, woT_sb, bo_sb,
                                 hacc, W)
                for oc in ([] if (SKIP_C or SKIP_L) else range(4)):
                    nc.sync.dma_start(
                        hT_out["L"][oc * 128:(oc + 1) * 128, :],
                        hacc[:, oc * W * 128:(oc + 1) * W * 128])

                # ---- AG#3: escore ----
                if not SKIP_AG and not SKIP_C:
                    nc.gpsimd.collective_compute(
                        "AllGather", mybir.AluOpType.bypass,
                        replica_groups=[list(range(NCORES))],
                        ins=[esc_sh], outs=[esc_full])

                # ---- phase C-R: right messages from shared scores ----
                for w in ([] if (SKIP_C or SKIP_R) else range(W)):
                    v = gpool.tile([128, BPW, C], bf16, tag="v")
                    nc.gpsimd.dma_gather(
                        v[:], tv_L[:, :], vidx_sb[:, w * 64:(w + 1) * 64],
                        WCAP, nidx_reg, C)
                    eg = kdpool.tile([128, BPW, 64], f32, tag="eg")
                    nc.gpsimd.dma_gather(
                        eg[:], esc_full[:, :], eidx_sb[:, w * 64:(w + 1) * 64],
                        WCAP, nidx_reg, 64)

                    # select each edge's eh via the host-shipped one-hot mask
                    egm = sp.tile([128, BPW, 64], bf16, tag="egm")
                    nc.vector.tensor_tensor(
                        egm[:], eg[:, :, :],
                        emask_sb[:, w * BPW * 64:(w + 1) * BPW * 64],
                        op=OP.mult)
                    ehR = sp.tile([128, BPW], f32, tag="ehR")
                    nc.vector.tensor_reduce(
                        ehR[:], egm[:, :, :],
                        axis=mybir.AxisListType.X, op=OP.add)

                    ohs = []
                    for b in range(BPW):
                        oh = ohpool.tile([128, 128], bf16, tag="oh")
                        nc.vector.tensor_scalar(
                            oh[:], iota_sb[:],
                            drelR_sb[:, w * BPW + b: w * BPW + b + 1],
                            ehR[:, b:b + 1], op0=OP.is_equal, op1=OP.mult)
                        ohs.append(oh)

                    msgT_ps = pmsg.tile([128, 512], f32)
                    z_ps = pz.tile([1, 128], f32)
                    for cc in range(4):
                        for b in range(BPW):
                            nc.tensor.matmul(
                                msgT_ps[:, cc * 128:(cc + 1) * 128],
                                lhsT=v[:, b, cc * 128:(cc + 1) * 128],
                                rhs=ohs[b][:],
                                start=(b == 0), stop=(b == BPW - 1))
                    for b in range(BPW):
                        nc.tensor.matmul(
                            z_ps[:], lhsT=ones_col[:], rhs=ohs[b][:],
                            start=(b == 0), stop=(b == BPW - 1))

                    _window_tail(nc, tc, mybir, w, msgT_ps, z_ps,
                                 tp, pzbc, ph, ones_row, woT_sb, bo_sb,
                                 hacc, W)
                for oc in ([] if (SKIP_C or SKIP_R) else range(4)):
                    nc.sync.dma_start(
                        hT_out["R"][oc * 128:(oc + 1) * 128, :],
                        hacc[:, oc * W * 128:(oc + 1) * W * 128])
    nc.compile()
    return nc


def _window_tail(nc, tc, mybir, w, msgT_ps, z_ps, tp, pzbc, ph,
                 ones_row, woT_sb, bo_sb, hacc, W):
    """z -> 1/z broadcast, msgT normalize, Wo GEMM, bias+LeakyReLU."""
    f32, bf16 = mybir.dt.float32, mybir.dt.bfloat16
    AF = mybir.ActivationFunctionType
    OP = mybir.AluOpType

    zm = tp.tile([1, 128], f32, tag="zm")
    nc.vector.tensor_scalar_max(zm[:], z_ps[:], 1e-30)
    zr = tp.tile([1, 128], f32, tag="zr")
    nc.vector.reciprocal(zr[:], zm[:])
    zrb = tp.tile([1, 128], bf16, tag="zrb")
    nc.vector.tensor_copy(zrb[:], zr[:])
    zbc_ps = pzbc.tile([128, 128], f32)
    nc.tensor.matmul(zbc_ps[:], lhsT=ones_row[:], rhs=zrb[:],
                     start=True, stop=True)
    zbc = tp.tile([128, 128], f32, tag="zbc")
    nc.scalar.copy(zbc[:], zbc_ps[:])
    msgT_sb = tp.tile([128, 512], bf16, tag="msgT")
    for nch in range(4):
        nc.vector.tensor_tensor(
            msgT_sb[:, nch * 128:(nch + 1) * 128],
            msgT_ps[:, nch * 128:(nch + 1) * 128],
            zbc[:], op=OP.mult)
    hT_ps = ph.tile([128, 512], f32)
    for oc in range(4):
        for cc in range(4):
            nc.tensor.matmul(
                hT_ps[:, oc * 128:(oc + 1) * 128],
                lhsT=woT_sb[:, cc * 512 + oc * 128:
                            cc * 512 + oc * 128 + 128],
                rhs=msgT_sb[:, cc * 128:(cc + 1) * 128],
                start=(cc == 0), stop=(cc == 3))
    for oc in range(4):
        x = hacc[:, (oc * W + w) * 128: (oc * W + w) * 128 + 128]
        nc.scalar.activation(
            x, hT_ps[:, oc * 128:(oc + 1) * 128],
            AF.Identity, bias=bo_sb[:, oc:oc + 1])
        x2 = tp.tile([128, 128], bf16, tag="x2")
        nc.vector.tensor_scalar_mul(x2[:], x, NEG)
        nc.vector.tensor_tensor(x, x, x2[:], op=OP.max)


def _assemble(results, arrs, key):
    out = np.zeros((N, C), np.float32)
    for c in range(NCORES):
        hT = np.asarray(results[c][key], np.float32)
        cn = arrs[c]["colnode"]
        m = cn >= 0
        out[cn[m]] = hT[:, m].T
    return out


_RUN_KWARGS = {}


def kernel(**inputs):
    from concourse.bass_utils import run_bass_kernel_spmd

    in_maps, arrL, arrR, W = _host_inputs(inputs)
    nc = _build_program(W)
    res = run_bass_kernel_spmd(nc, in_maps, core_ids=list(range(NCORES)),
                               **_RUN_KWARGS)
    out_l = _assemble(res.results, arrL, "hT_L")
    out_r = _assemble(res.results, arrR, "hT_R")
    kernel.last_results = res
    kernel.last_nc = nc
    kernel.last_W = W
    return (out_l, out_r)


# revision 23
# speedup vs baseline: 1.2940x; 1.1230x over previous
"""Trainium2 Bass kernel for nn_MultiHeadAttention_73589969649754
(gnn_message_passing / graph cross-attention).

v3 strategy (score sharing + matmul scoring):
  - Edges sorted by destination per side; core c owns nodes
    [c*2500, (c+1)*2500) on both sides.  Windows of <=128 consecutive
    nodes / <=1024 edge slots.  The HOST permutes each core's node
    columns so window w occupies slots [w*128, (w+1)*128) - all program
    offsets are SPMD-uniform and tables are slot-ordered.
  - Phase A: R side emits a fused K|V table (AllGather #1, 5.24MB/rank);
    L side computes K TRANSPOSED ([channel, slot], kept in SBUF - no
    DRAM round trip) and a V table (AllGather #2, 2.62MB/rank).
  - L pass per window: one TRANSPOSED dma_gather pulls Kr[src] in
    [channel, edge] orientation and a normal gather pulls Vr[src].
    Scores come from PE matmuls M[slot, edge] = KlT_win^T @ KrT_gath
    (no per-edge dot product on DVE/ACT), then exp on ACT, a host-shipped
    one-hot mask zeroes off-segment entries, and a PE transpose yields
    E^T[edge, slot] - which IS the eh-scaled one-hot the segment-sum
    matmuls consume.  eh per edge (row-sum of E^T) is written to a score
    shard (AllGather #3, 80KB/rank).  z via ones-matmul; messages are
    normalized after the fact by 1/z (column broadcast); Wo GEMM;
    bias+LeakyReLU.
  - R pass: gathers only Vl[src] rows (1KB/edge) plus 256B score-table
    rows; a host-shipped one-hot mask selects each edge's eh (softmax
    numerators are shared between the two sides).  No K gather, no dot
    product, no exp on the R side.
  - Outputs stored bf16 in [channel, slot] layout; host reassembles.
"""

import math

import numpy as np

N = 20000
E = 160000
C = 512
NCORES = 8
TEMP = float(np.sqrt(C))
NEG = 0.01
NPC = N // NCORES            # 2500 nodes per core per side
BLK = 128                    # edges per block
BPW = 8                      # blocks per window
WCAP = BPW * BLK             # 1024 edge slots per window
DUMMY_REL = 999.0
SKIP_AG = False
SKIP_C = False
SKIP_L = False       # debug: skip the L window loop
SKIP_R = False       # debug: skip the R window loop

# AllGather wall-time charge (ns): measured-table upper bound is ~70us for
# a 5.24MB/rank 8-way intra-chip AllGather; scale by bytes with a 20us
# latency floor.  AG#1 tkv_R 5.24MB -> 70us, AG#2 tv_L 2.62MB -> 35us,
# AG#3 escore 80KB -> 20us (floor).
AG_CHARGE_NS = 70e3 + 35e3 + 20e3


def _prep_side(seg_dst):
    """Sort edges by dst; node-aligned core ranges; pack windows."""
    seg_dst = np.asarray(seg_dst, np.int64)
    perm = np.argsort(seg_dst, kind="stable")
    sd = seg_dst[perm]
    deg = np.bincount(sd, minlength=N)
    edge_b = [int(np.searchsorted(sd, c * NPC, "left"))
              for c in range(NCORES)] + [E]

    cores = []
    max_w = 0
    for c in range(NCORES):
        n0, n1 = c * NPC, (c + 1) * NPC
        e0 = edge_b[c]
        wins = []
        n, e = n0, e0
        while n < n1:
            wn = we = 0
            while n + wn < n1 and wn < BLK and we + deg[n + wn] <= WCAP:
                we += deg[n + wn]
                wn += 1
            assert wn > 0, "node degree exceeds window capacity"
            wins.append((n, wn, e, we))
            n += wn
            e += we
        assert e == edge_b[c + 1]
        cores.append((wins, e0))
        max_w = max(max_w, len(wins))
    return perm, sd, cores, max_w


def _wrap_idx16(idx_flat):
    """[n] -> [128, n//16] int16, i at [i%16, i//16], replicated x8."""
    n = idx_flat.shape[0]
    a = idx_flat.reshape(n // 16, 16).T.astype(np.int16)
    return np.ascontiguousarray(np.tile(a, (8, 1)))


def _slot_maps(cores, W):
    """Slot-order the nodes: window w of core c occupies slots
    [w*128, (w+1)*128).  Returns (slot_node [NCORES, W*128] node-or--1,
    node_slot [N] global slot = core*W*128 + slot)."""
    npad = W * BLK
    slot_node = np.full((NCORES, npad), -1, np.int64)
    node_slot = np.full(N, -1, np.int64)
    for c, (wins, e0) in enumerate(cores):
        for w, (fn, wn, es, ne) in enumerate(wins):
            sl = np.arange(wn)
            slot_node[c, w * BLK + sl] = fn + sl
            node_slot[fn + sl] = c * npad + w * BLK + sl
    assert (node_slot >= 0).all()
    return slot_node, node_slot


def _edge_locs(perm, cores):
    """Per original edge id: (core, window, slot-in-window) on this side."""
    core_of = np.empty(E, np.int32)
    win_of = np.empty(E, np.int32)
    slot_of = np.empty(E, np.int32)
    for c, (wins, e0) in enumerate(cores):
        for w, (fn, wn, es, ne) in enumerate(wins):
            ids = perm[es:es + ne]
            core_of[ids] = c
            win_of[ids] = w
            slot_of[ids] = np.arange(ne)
    return core_of, win_of, slot_of


def _host_inputs(inputs):
    import ml_dtypes
    bf16 = ml_dtypes.bfloat16

    nl = np.asarray(inputs["node_left"], np.float32)
    nr = np.asarray(inputs["node_right"], np.float32)
    Wk = np.asarray(inputs["Wk"], np.float32)
    Wv = np.asarray(inputs["Wv"], np.float32)
    Wo = np.asarray(inputs["Wo"], np.float32)
    bo = np.asarray(inputs["bo"], np.float32)
    sl = np.asarray(inputs["segmentation_index_left"], np.int64)
    sr = np.asarray(inputs["segmentation_index_right"], np.int64)

    permL, sdL, coresL, wL = _prep_side(sl)
    permR, sdR, coresR, wR = _prep_side(sr)
    W = max(wL, wR)
    npad = W * BLK

    snL, nsL = _slot_maps(coresL, W)     # left-node slots
    snR, nsR = _slot_maps(coresR, W)     # right-node slots
    LcoreE, LwinE, LslotE = _edge_locs(permL, coresL)

    # ---- per-core L arrays ----
    ssL = sr[permL]          # src (right) node per L-sorted edge
    arrL = []
    for c, (wins, e0) in enumerate(coresL):
        sidx = np.zeros((W, WCAP), np.int64)      # src rows in R table
        drel = np.full((W, WCAP), DUMMY_REL, np.float32)
        qtm = np.zeros((BLK, W * WCAP), np.float32)
        for w, (fn, wn, es, ne) in enumerate(wins):
            sidx[w, :ne] = nsR[ssL[es:es + ne]]
            rel = (sdL[es:es + ne] - fn)
            drel[w, :ne] = rel.astype(np.float32)
            # QT mask: [dst-rel, (w*8+b)*128 + p] = 1 for edge slot i=b*128+p
            i = np.arange(ne)
            qtm[rel, w * WCAP + (i // BLK) * BLK + (i % BLK)] = 1.0
        sidx16 = np.concatenate([_wrap_idx16(sidx[w]) for w in range(W)], 1)
        drelT = np.ascontiguousarray(
            drel.reshape(W, BPW, BLK).transpose(2, 0, 1).reshape(BLK, W * BPW))
        arrL.append(dict(sidx=sidx16, drel=drelT, qtm=qtm.astype(bf16),
                         colnode=snL[c]))

    # ---- per-core R arrays ----
    ssR = sl[permR]          # src (left) node per R-sorted edge
    arrR = []
    for c, (wins, e0) in enumerate(coresR):
        vidx = np.zeros((W, WCAP), np.int64)      # src rows in L V table
        eidx = np.zeros((W, WCAP), np.int64)      # escore row (64-col rows)
        ecol = np.zeros((W, WCAP), np.int64)
        emask_valid = np.zeros((W, WCAP), bool)
        drel = np.full((W, WCAP), DUMMY_REL, np.float32)
        for w, (fn, wn, es, ne) in enumerate(wins):
            ids = permR[es:es + ne]
            vidx[w, :ne] = nsL[ssR[es:es + ne]]
            # flat L escore position: core*(W*1024) + win*1024 + p*8 + b
            pL = LslotE[ids] % BLK
            bL = LslotE[ids] // BLK
            gpos = (LcoreE[ids].astype(np.int64) * W + LwinE[ids]) * WCAP \
                + pL * BPW + bL
            eidx[w, :ne] = gpos // 64
            ecol[w, :ne] = gpos % 64
            emask_valid[w, :ne] = True
            drel[w, :ne] = (sdR[es:es + ne] - fn).astype(np.float32)
        vidx16 = np.concatenate([_wrap_idx16(vidx[w]) for w in range(W)], 1)
        eidx16 = np.concatenate([_wrap_idx16(eidx[w]) for w in range(W)], 1)
        drelT = np.ascontiguousarray(
            drel.reshape(W, BPW, BLK).transpose(2, 0, 1).reshape(BLK, W * BPW))
        emask = np.zeros((BLK, W * BPW * 64), np.float32)
        wv, iv = np.nonzero(emask_valid)
        pv, bv = iv % BLK, iv // BLK
        emask[pv, (wv * BPW + bv) * 64 + ecol[wv, iv]] = 1.0
        arrR.append(dict(vidx=vidx16, eidx=eidx16, drel=drelT,
                         emask=emask.astype(bf16), colnode=snR[c]))

    # ---- shared constants ----
    Wkv = np.concatenate([Wk, Wv], 0)               # [1024, 512]
    WkvT = Wkv.T                                    # [512, 1024]
    wkvT_arr = np.zeros((128, 4 * 1024), np.float32)
    for cc in range(4):
        wkvT_arr[:, cc * 1024:(cc + 1) * 1024] = \
            WkvT[cc * 128:(cc + 1) * 128, :]
    # wkT: lhsT tiles for the transposed K GEMM
    wkT_arr = np.zeros((128, 16 * 128), np.float32)
    for i in range(4):
        for o in range(4):
            wkT_arr[:, (i * 4 + o) * 128:(i * 4 + o + 1) * 128] = \
                Wk[o * 128:(o + 1) * 128, i * 128:(i + 1) * 128].T
    woT_arr = np.zeros((128, 4 * 512), np.float32)
    for cc in range(4):
        for oc in range(4):
            woT_arr[:, cc * 512 + oc * 128: cc * 512 + (oc + 1) * 128] = \
                Wo[oc * 128:(oc + 1) * 128, cc * 128:(cc + 1) * 128].T
    bo_arr = bo.reshape(4, 128).T.copy()            # [128, 4]
    iota_arr = np.broadcast_to(
        np.arange(128, dtype=np.float32)[None, :], (128, 128))
    ident_arr = np.eye(128, dtype=np.float32)

    def shardT(feat, slot_node_c):
        sh = np.zeros((C, npad), np.float32)
        m = slot_node_c >= 0
        sh[:, m] = feat[slot_node_c[m]].T
        return np.ascontiguousarray(sh).astype(bf16)

    in_maps = []
    for c in range(NCORES):
        in_maps.append({
            "nT_L": shardT(nl, snL[c]),
            "nT_R": shardT(nr, snR[c]),
            "wkvT": wkvT_arr.astype(bf16),
            "wkT": wkT_arr.astype(bf16),
            "woT": woT_arr.astype(bf16),
            "bo": bo_arr,
            "iota": np.ascontiguousarray(iota_arr).astype(bf16),
            "ident": np.ascontiguousarray(ident_arr).astype(bf16),
            "sidx_L": arrL[c]["sidx"],
            "drel_L": arrL[c]["drel"],
            "qtm_L": arrL[c]["qtm"],
            "vidx_R": arrR[c]["vidx"],
            "eidx_R": arrR[c]["eidx"],
            "drel_R": arrR[c]["drel"],
            "emask_R": arrR[c]["emask"],
        })
    return in_maps, arrL, arrR, W


def _build_program(W):
    import concourse.bacc as bacc
    import concourse.tile as tile
    from concourse import mybir

    dt = mybir.dt
    f32, bf16, i16 = dt.float32, dt.bfloat16, dt.int16
    AF = mybir.ActivationFunctionType
    OP = mybir.AluOpType
    npad = W * BLK
    ntiles = W

    nc = bacc.Bacc("TRN2", target_bir_lowering=False, debug=False,
                   enable_asserts=True, num_devices=NCORES,
                   num_swdge_queues=2)

    # ---- I/O ----
    nT_in = {s: nc.dram_tensor(f"nT_{s}", [C, npad], bf16,
                               kind="ExternalInput").ap() for s in "LR"}
    wkvT = nc.dram_tensor("wkvT", [128, 4 * 1024], bf16,
                          kind="ExternalInput").ap()
    wkT_in = nc.dram_tensor("wkT", [128, 16 * 128], bf16,
                            kind="ExternalInput").ap()
    woT = nc.dram_tensor("woT", [128, 4 * 512], bf16,
                         kind="ExternalInput").ap()
    bo_in = nc.dram_tensor("bo", [128, 4], f32, kind="ExternalInput").ap()
    iota_in = nc.dram_tensor("iota", [128, 128], bf16,
                             kind="ExternalInput").ap()
    ident_in = nc.dram_tensor("ident", [128, 128], bf16,
                              kind="ExternalInput").ap()
    sidx_in = nc.dram_tensor("sidx_L", [128, W * 64], i16,
                             kind="ExternalInput").ap()
    drelL_in = nc.dram_tensor("drel_L", [128, W * BPW], f32,
                              kind="ExternalInput").ap()
    qtm_in = nc.dram_tensor("qtm_L", [128, W * WCAP], bf16,
                            kind="ExternalInput").ap()
    vidx_in = nc.dram_tensor("vidx_R", [128, W * 64], i16,
                             kind="ExternalInput").ap()
    eidx_in = nc.dram_tensor("eidx_R", [128, W * 64], i16,
                             kind="ExternalInput").ap()
    drelR_in = nc.dram_tensor("drel_R", [128, W * BPW], f32,
                              kind="ExternalInput").ap()
    emask_in = nc.dram_tensor("emask_R", [128, W * BPW * 64], bf16,
                              kind="ExternalInput").ap()
    hT_out = {s: nc.dram_tensor(f"hT_{s}", [C, npad], bf16,
                                kind="ExternalOutput").ap() for s in "LR"}

    # ---- internal DRAM ----
    tkv_shR = nc.dram_tensor("tkv_shR", [npad, 2 * C], bf16).ap()
    tv_shL = nc.dram_tensor("tv_shL", [npad, C], bf16).ap()
    esc_sh = nc.dram_tensor("esc_sh", [W, 128, BPW], f32).ap()
    tkv_R = nc.dram_tensor("tkv_R", [NCORES * npad, 2 * C], bf16,
                           addr_space="Shared").ap()
    tv_L = nc.dram_tensor("tv_L", [NCORES * npad, C], bf16,
                          addr_space="Shared").ap()
    esc_full = nc.dram_tensor("esc_full", [NCORES * W * 16, 64], f32,
                              addr_space="Shared").ap()

    with tile.TileContext(nc) as tc:
        with tc.tile_pool(name="const", bufs=1) as cpool:
            # early constants (phase A + L loop)
            wkvT_sb = cpool.tile([128, 4 * 1024], bf16)
            nc.sync.dma_start(wkvT_sb[:], wkvT[:, :])
            wkT_sb = cpool.tile([128, 16 * 128], bf16)
            nc.sync.dma_start(wkT_sb[:], wkT_in[:, :])
            sidx_sb = cpool.tile([128, W * 64], i16)
            nc.sync.dma_start(sidx_sb[:], sidx_in[:, :])
            drelL_sb = cpool.tile([128, W * BPW], f32)
            nc.sync.dma_start(drelL_sb[:], drelL_in[:, :])
            ident_sb = cpool.tile([128, 128], bf16)
            nc.sync.dma_start(ident_sb[:], ident_in[:, :])
            woT_sb = cpool.tile([128, 4 * 512], bf16)
            nc.sync.dma_start(woT_sb[:], woT[:, :])
            bo_sb = cpool.tile([128, 4], f32)
            nc.sync.dma_start(bo_sb[:], bo_in[:, :])
            iota_sb = cpool.tile([128, 128], bf16)
            nc.sync.dma_start(iota_sb[:], iota_in[:, :])
            ones_col = cpool.tile([128, 1], bf16)
            nc.vector.memset(ones_col[:], 1.0)
            ones_row = cpool.tile([1, 128], bf16)
            nc.vector.memset(ones_row[:], 1.0)
            # late constants (R loop only; loaded after phase A issues)
            vidx_sb = cpool.tile([128, W * 64], i16)
            eidx_sb = cpool.tile([128, W * 64], i16)
            drelR_sb = cpool.tile([128, W * BPW], f32)
            emask_sb = cpool.tile([128, W * BPW * 64], bf16)
            hacc = cpool.tile([128, 4 * npad], bf16)
            klT_sb = cpool.tile([128, 4, npad], bf16)   # left K transposed

            # ---- phase A ----
            with (
                tc.tile_pool(name="feat", bufs=1) as fpool,
                tc.tile_pool(name="gemm_sb", bufs=3) as gsb,
                tc.tile_pool(name="psum_gemm", bufs=2, space="PSUM") as pg,
            ):
                featR = []
                for cc in range(4):
                    t = fpool.tile([128, npad], bf16, tag=f"featR{cc}")
                    nc.sync.dma_start(
                        t[:], nT_in["R"][cc * 128:(cc + 1) * 128, :])
                    featR.append(t)
                # R side K|V fused GEMM -> tkv_shR -> AG#1
                for ti in range(ntiles):
                    sb = gsb.tile([128, 1024], bf16)
                    for half in range(2):
                        ps = pg.tile([128, 512], f32)
                        for cc in range(4):
                            nc.tensor.matmul(
                                ps[:],
                                lhsT=featR[cc][:, ti * 128:(ti + 1) * 128],
                                rhs=wkvT_sb[:, cc * 1024 + half * 512:
                                            cc * 1024 + half * 512 + 512],
                                start=(cc == 0), stop=(cc == 3))
                        nc.vector.tensor_copy(
                            sb[:, half * 512:(half + 1) * 512], ps[:])
                    nc.sync.dma_start(
                        tkv_shR[ti * 128:(ti + 1) * 128, :], sb[:])
                if not SKIP_AG:
                    nc.gpsimd.collective_compute(
                        "AllGather", mybir.AluOpType.bypass,
                        replica_groups=[list(range(NCORES))],
                        ins=[tkv_shR], outs=[tkv_R])
                else:
                    # timing build: sliver copy keeps the dependency edge
                    nc.sync.dma_start(tkv_R[0:128, :], tkv_shR[0:128, :])

                featL = []
                for cc in range(4):
                    t = fpool.tile([128, npad], bf16, tag=f"featL{cc}")
                    nc.sync.dma_start(
                        t[:], nT_in["L"][cc * 128:(cc + 1) * 128, :])
                    featL.append(t)
                # L side transposed-K GEMM -> klT_sb (stays in SBUF)
                for o in range(4):
                    for nb in range(math.ceil(npad / 512)):
                        n0 = nb * 512
                        n1 = min(npad, n0 + 512)
                        ps = pg.tile([128, 512], f32)
                        for i in range(4):
                            nc.tensor.matmul(
                                ps[:, 0:n1 - n0],
                                lhsT=wkT_sb[:, (i * 4 + o) * 128:
                                            (i * 4 + o + 1) * 128],
                                rhs=featL[i][:, n0:n1],
                                start=(i == 0), stop=(i == 3))
                        nc.vector.tensor_copy(
                            klT_sb[:, o, n0:n1], ps[:, 0:n1 - n0])
                # L side V GEMM -> tv_shL -> AG#2
                for ti in range(ntiles):
                    sb = gsb.tile([128, 512], bf16, tag="sbv")
                    ps = pg.tile([128, 512], f32)
                    for cc in range(4):
                        nc.tensor.matmul(
                            ps[:],
                            lhsT=featL[cc][:, ti * 128:(ti + 1) * 128],
                            rhs=wkvT_sb[:, cc * 1024 + 512:
                                        cc * 1024 + 1024],
                            start=(cc == 0), stop=(cc == 3))
                    nc.vector.tensor_copy(sb[:], ps[:])
                    nc.sync.dma_start(
                        tv_shL[ti * 128:(ti + 1) * 128, :], sb[:])
                if not SKIP_AG:
                    nc.gpsimd.collective_compute(
                        "AllGather", mybir.AluOpType.bypass,
                        replica_groups=[list(range(NCORES))],
                        ins=[tv_shL], outs=[tv_L])
                else:
                    nc.sync.dma_start(tv_L[0:128, :], tv_shL[0:128, :])

            # late const loads (R loop)
            nc.sync.dma_start(vidx_sb[:], vidx_in[:, :])
            nc.sync.dma_start(eidx_sb[:], eidx_in[:, :])
            nc.sync.dma_start(drelR_sb[:], drelR_in[:, :])
            nc.sync.dma_start(emask_sb[:], emask_in[:, :])

            nidx_reg = nc.gpsimd.to_reg(WCAP)
            nidx_reg2 = nc.gpsimd.to_reg(WCAP // 2)

            with (
                tc.tile_pool(name="gath", bufs=3) as gpool,
                tc.tile_pool(name="aux2", bufs=3) as kdpool,
                tc.tile_pool(name="qtm", bufs=2) as qpool,
                tc.tile_pool(name="blk", bufs=4) as sp,
                tc.tile_pool(name="ebuf", bufs=3) as ebpool,
                tc.tile_pool(name="etbuf", bufs=2 * BPW) as ohpool,
                tc.tile_pool(name="tail", bufs=3) as tp,
                tc.tile_pool(name="pMT", bufs=2, space="PSUM") as pMT,
                tc.tile_pool(name="pmsg", bufs=2, space="PSUM") as pmsg,
                tc.tile_pool(name="paux", bufs=1, space="PSUM") as paux,
                tc.tile_pool(name="ph", bufs=1, space="PSUM") as ph,
            ):
                # ---- phase C-L: matmul scores + left messages ----
                for w in ([] if (SKIP_C or SKIP_L) else range(W)):
                    # transposed K gather split in two: the worker's ucode
                    # fails above 512 indices per transpose gather
                    ktgs = []
                    for h in range(2):
                        kt = gpool.tile([128, 4, WCAP // 2], bf16,
                                        tag=f"ktg{h}")
                        nc.gpsimd.dma_gather(
                            kt[:], tkv_R[:, 0:C],
                            sidx_sb[:, w * 64 + h * 32: w * 64 + h * 32 + 32],
                            WCAP // 2, nidx_reg2, C, elem_step=2 * C,
                            transpose=True)
                        ktgs.append(kt)
                    vg = gpool.tile([128, BPW, C], bf16, tag="vg")
                    nc.gpsimd.dma_gather(
                        vg[:], tkv_R[:, C:2 * C],
                        sidx_sb[:, w * 64:(w + 1) * 64],
                        WCAP, nidx_reg, C, elem_step=2 * C, queue_num=1)
                    qtm = qpool.tile([128, WCAP], bf16, tag="qtm")
                    nc.sync.dma_start(
                        qtm[:], qtm_in[:, w * WCAP:(w + 1) * WCAP])

                    ehs = sp.tile([128, BPW], f32, tag="ehs")
                    ets = []
                    msgT_ps = pmsg.tile([128, 512], f32)
                    z_ps = paux.tile([128, 128], f32, tag="aux")
                    for b in range(BPW):
                        M_ps = pMT.tile([128, 128], f32, tag="M")
                        kth = ktgs[b // 4]
                        bh = b % 4
                        for j in range(4):
                            nc.tensor.matmul(
                                M_ps[:],
                                lhsT=klT_sb[:, j, w * 128:(w + 1) * 128],
                                rhs=kth[:, j, bh * 128:(bh + 1) * 128],
                                start=(j == 0), stop=(j == 3))
                        eb = ebpool.tile([128, 128], bf16, tag="eb")
                        nc.scalar.activation(eb[:], M_ps[:], AF.Exp,
                                             scale=1.0 / TEMP)
                        em = ebpool.tile([128, 128], bf16, tag="em")
                        nc.vector.tensor_tensor(
                            em[:], eb[:],
                            qtm[:, b * 128:(b + 1) * 128], op=OP.mult)
                        T_ps = pMT.tile([128, 128], bf16, tag="T")
                        nc.tensor.transpose(T_ps[:], em[:], ident_sb[:])
                        et = ohpool.tile([128, 128], bf16, tag="et")
                        nc.scalar.copy(et[:], T_ps[:])
                        # eh per edge = row-sum of E^T (raw, pre-normalize)
                        nc.vector.tensor_reduce(
                            ehs[:, b:b + 1], et[:],
                            axis=mybir.AxisListType.X, op=OP.add)
                        ets.append(et)
                    # escore shard write: esc[w, p, b] = eh[p, b]
                    nc.sync.dma_start(esc_sh[w, :, :], ehs[:])

                    for cc in range(4):
                        for b in range(BPW):
                            nc.tensor.matmul(
                                msgT_ps[:, cc * 128:(cc + 1) * 128],
                                lhsT=vg[:, b, cc * 128:(cc + 1) * 128],
                                rhs=ets[b][:],
                                start=(b == 0), stop=(b == BPW - 1))
                    for b in range(BPW):
                        nc.tensor.matmul(
                            z_ps[0:1, :], lhsT=ones_col[:], rhs=ets[b][:],
                            start=(b == 0), stop=(b == BPW - 1))

                    _window_tail(nc, tc, mybir, w, msgT_ps, z_ps,
                                 tp, paux, ph, ones_row, woT_sb, bo_sb,
                                 hacc, W)
                for oc in ([] if (SKIP_C or SKIP_L) else range(4)):
                    nc.sync.dma_start(
                        hT_out["L"][oc * 128:(oc + 1) * 128, :],
                        hacc[:, oc * npad:(oc + 1) * npad])

                # ---- AG#3: escore ----
                if not SKIP_C:
                    if not SKIP_AG:
                        nc.gpsimd.collective_compute(
                            "AllGather", mybir.AluOpType.bypass,
                            replica_groups=[list(range(NCORES))],
                            ins=[esc_sh], outs=[esc_full])
                    else:
                        nc.sync.dma_start(esc_full[0:2, :],
                                          esc_sh[0, 0:16, :])

                # ---- phase C-R: right messages from shared scores ----
                for w in ([] if (SKIP_C or SKIP_R) else range(W)):
                    v = gpool.tile([128, BPW, C], bf16, tag="v")
                    nc.gpsimd.dma_gather(
                        v[:], tv_L[:, :], vidx_sb[:, w * 64:(w + 1) * 64],
                        WCAP, nidx_reg, C)
                    eg = kdpool.tile([128, BPW, 64], f32, tag="eg")
                    nc.gpsimd.dma_gather(
                        eg[:], esc_full[:, :], eidx_sb[:, w * 64:(w + 1) * 64],
                        WCAP, nidx_reg, 64, queue_num=1)

                    # select each edge's eh via the host-shipped one-hot mask
                    egm = sp.tile([128, BPW, 64], bf16, tag="egm")
                    nc.gpsimd.tensor_tensor(
                        egm[:], eg[:, :, :],
                        emask_sb[:, w * BPW * 64:(w + 1) * BPW * 64],
                        op=OP.mult)
                    ehR = sp.tile([128, BPW], f32, tag="ehR")
                    nc.vector.tensor_reduce(
                        ehR[:], egm[:, :, :],
                        axis=mybir.AxisListType.X, op=OP.add)

                    ohs = []
                    for b in range(BPW):
                        oh = ohpool.tile([128, 128], bf16, tag="et")
                        nc.vector.tensor_scalar(
                            oh[:], iota_sb[:],
                            drelR_sb[:, w * BPW + b: w * BPW + b + 1],
                            ehR[:, b:b + 1], op0=OP.is_equal, op1=OP.mult)
                        ohs.append(oh)

                    msgT_ps = pmsg.tile([128, 512], f32)
                    z_ps = paux.tile([128, 128], f32, tag="aux")
                    for cc in range(4):
                        for b in range(BPW):
                            nc.tensor.matmul(
                                msgT_ps[:, cc * 128:(cc + 1) * 128],
                                lhsT=v[:, b, cc * 128:(cc + 1) * 128],
                                rhs=ohs[b][:],
                                start=(b == 0), stop=(b == BPW - 1))
                    for b in range(BPW):
                        nc.tensor.matmul(
                            z_ps[0:1, :], lhsT=ones_col[:], rhs=ohs[b][:],
                            start=(b == 0), stop=(b == BPW - 1))

                    _window_tail(nc, tc, mybir, w, msgT_ps, z_ps,
                                 tp, paux, ph, ones_row, woT_sb, bo_sb,
                                 hacc, W)
                for oc in ([] if (SKIP_C or SKIP_R) else range(4)):
                    nc.sync.dma_start(
                        hT_out["R"][oc * 128:(oc + 1) * 128, :],
                        hacc[:, oc * npad:(oc + 1) * npad])
    nc.compile()
    return nc


def _window_tail(nc, tc, mybir, w, msgT_ps, z_ps, tp, paux, ph,
                 ones_row, woT_sb, bo_sb, hacc, W):
    """z -> 1/z broadcast, msgT normalize, Wo GEMM, bias+LeakyReLU."""
    f32, bf16 = mybir.dt.float32, mybir.dt.bfloat16
    AF = mybir.ActivationFunctionType
    OP = mybir.AluOpType
    npad = W * 128

    zm = tp.tile([1, 128], f32, tag="zm")
    nc.vector.tensor_scalar_max(zm[:], z_ps[0:1, :], 1e-30)
    zr = tp.tile([1, 128], f32, tag="zr")
    nc.vector.reciprocal(zr[:], zm[:])
    zrb = tp.tile([1, 128], bf16, tag="zrb")
    nc.vector.tensor_copy(zrb[:], zr[:])
    zbc_ps = paux.tile([128, 128], f32, tag="aux")
    nc.tensor.matmul(zbc_ps[:], lhsT=ones_row[:], rhs=zrb[:],
                     start=True, stop=True)
    zbc = tp.tile([128, 128], f32, tag="zbc")
    nc.scalar.copy(zbc[:], zbc_ps[:])
    msgT_sb = tp.tile([128, 512], bf16, tag="msgT")
    for nch in range(4):
        nc.vector.tensor_tensor(
            msgT_sb[:, nch * 128:(nch + 1) * 128],
            msgT_ps[:, nch * 128:(nch + 1) * 128],
            zbc[:], op=OP.mult)
    hT_ps = ph.tile([128, 512], f32)
    for oc in range(4):
        for cc in range(4):
            nc.tensor.matmul(
                hT_ps[:, oc * 128:(oc + 1) * 128],
                lhsT=woT_sb[:, cc * 512 + oc * 128:
                            cc * 512 + oc * 128 + 128],
                rhs=msgT_sb[:, cc * 128:(cc + 1) * 128],
                start=(cc == 0), stop=(cc == 3))
    for oc in range(4):
        x = hacc[:, oc * npad + w * 128: oc * npad + (w + 1) * 128]
        nc.scalar.activation(
            x, hT_ps[:, oc * 128:(oc + 1) * 128],
            AF.Identity, bias=bo_sb[:, oc:oc + 1])
        x2 = tp.tile([128, 128], bf16, tag="x2")
        nc.vector.tensor_scalar_mul(x2[:], x, NEG)
        nc.vector.tensor_tensor(x, x, x2[:], op=OP.max)


def _assemble(results, arrs, key):
    out = np.zeros((N, C), np.float32)
    for c in range(NCORES):
        hT = np.asarray(results[c][key], np.float32)
        cn = arrs[c]["colnode"]
        m = cn >= 0
        out[cn[m]] = hT[:, m].T
    return out


_RUN_KWARGS = {}


def kernel(**inputs):
    from concourse.bass_utils import run_bass_kernel_spmd

    in_maps, arrL, arrR, W = _host_inputs(inputs)
    nc = _build_program(W)
    res = run_bass_kernel_spmd(nc, in_maps, core_ids=list(range(NCORES)),
                               **_RUN_KWARGS)
    out_l = _assemble(res.results, arrL, "hT_L")
    out_r = _assemble(res.results, arrR, "hT_R")
    kernel.last_results = res
    kernel.last_nc = nc
    kernel.last_W = W
    return (out_l, out_r)


# revision 26
# speedup vs baseline: 1.3914x; 1.0752x over previous
"""Trainium2 Bass kernel for nn_MultiHeadAttention_73589969649754
(gnn_message_passing / graph cross-attention).

v3 strategy (score sharing + matmul scoring):
  - Edges sorted by destination per side; core c owns nodes
    [c*2500, (c+1)*2500) on both sides.  Windows of <=128 consecutive
    nodes / <=1024 edge slots.  The HOST permutes each core's node
    columns so window w occupies slots [w*128, (w+1)*128) - all program
    offsets are SPMD-uniform and tables are slot-ordered.
  - Phase A: R side emits a fused K|V table (AllGather #1, 5.24MB/rank);
    L side computes K TRANSPOSED ([channel, slot], kept in SBUF - no
    DRAM round trip) and a V table (AllGather #2, 2.62MB/rank).
  - L pass per window: one TRANSPOSED dma_gather pulls Kr[src] in
    [channel, edge] orientation and a normal gather pulls Vr[src].
    Scores come from PE matmuls M[slot, edge] = KlT_win^T @ KrT_gath
    (no per-edge dot product on DVE/ACT), then exp on ACT, a host-shipped
    one-hot mask zeroes off-segment entries, and a PE transpose yields
    E^T[edge, slot] - which IS the eh-scaled one-hot the segment-sum
    matmuls consume.  eh per edge (row-sum of E^T) is written to a score
    shard (AllGather #3, 80KB/rank).  z via ones-matmul; messages are
    normalized after the fact by 1/z (column broadcast); Wo GEMM;
    bias+LeakyReLU.
  - R pass: gathers only Vl[src] rows (1KB/edge) plus 256B score-table
    rows; a host-shipped one-hot mask selects each edge's eh (softmax
    numerators are shared between the two sides).  No K gather, no dot
    product, no exp on the R side.
  - Outputs stored bf16 in [channel, slot] layout; host reassembles.
"""

import math

import numpy as np

N = 20000
E = 160000
C = 512
NCORES = 8
TEMP = float(np.sqrt(C))
NEG = 0.01
NPC = N // NCORES            # 2500 nodes per core per side
BLK = 128                    # edges per block
BPW = 8                      # blocks per window
WCAP = BPW * BLK             # 1024 edge slots per window
DUMMY_REL = 999.0
SKIP_AG = False
SKIP_C = False
SKIP_L = False       # debug: skip the L window loop
SKIP_R = False       # debug: skip the R window loop

# AllGather wall-time charge (ns): measured-table upper bound is ~70us for
# a 5.24MB/rank 8-way intra-chip AllGather; scale by bytes with a 20us
# latency floor.  Serial charges: AG#1 tkv_R 5.24MB -> 70us (gates the L
# loop) and AG#3 escore 80KB -> 20us (floor; gates the R score gathers).
# AG#2 (tv_L, 2.62MB -> ~35us) runs on the collective cores/links right
# after AG#1 (~105us done) and is only consumed by the R loop, which
# starts after the ~175us L loop - fully hidden, so not charged.
AG_CHARGE_NS = 70e3 + 20e3


def _prep_side(seg_dst):
    """Sort edges by dst; node-aligned core ranges; pack windows."""
    seg_dst = np.asarray(seg_dst, np.int64)
    perm = np.argsort(seg_dst, kind="stable")
    sd = seg_dst[perm]
    deg = np.bincount(sd, minlength=N)
    edge_b = [int(np.searchsorted(sd, c * NPC, "left"))
              for c in range(NCORES)] + [E]

    cores = []
    max_w = 0
    for c in range(NCORES):
        n0, n1 = c * NPC, (c + 1) * NPC
        e0 = edge_b[c]
        wins = []
        n, e = n0, e0
        while n < n1:
            wn = we = 0
            while n + wn < n1 and wn < BLK and we + deg[n + wn] <= WCAP:
                we += deg[n + wn]
                wn += 1
            assert wn > 0, "node degree exceeds window capacity"
            wins.append((n, wn, e, we))
            n += wn
            e += we
        assert e == edge_b[c + 1]
        cores.append((wins, e0))
        max_w = max(max_w, len(wins))
    return perm, sd, cores, max_w


def _wrap_idx16(idx_flat):
    """[n] -> [128, n//16] int16, i at [i%16, i//16], replicated x8."""
    n = idx_flat.shape[0]
    a = idx_flat.reshape(n // 16, 16).T.astype(np.int16)
    return np.ascontiguousarray(np.tile(a, (8, 1)))


def _slot_maps(cores, W):
    """Slot-order the nodes: window w of core c occupies slots
    [w*128, (w+1)*128).  Returns (slot_node [NCORES, W*128] node-or--1,
    node_slot [N] global slot = core*W*128 + slot)."""
    npad = W * BLK
    slot_node = np.full((NCORES, npad), -1, np.int64)
    node_slot = np.full(N, -1, np.int64)
    for c, (wins, e0) in enumerate(cores):
        for w, (fn, wn, es, ne) in enumerate(wins):
            sl = np.arange(wn)
            slot_node[c, w * BLK + sl] = fn + sl
            node_slot[fn + sl] = c * npad + w * BLK + sl
    assert (node_slot >= 0).all()
    return slot_node, node_slot


def _edge_locs(perm, cores):
    """Per original edge id: (core, window, slot-in-window) on this side."""
    core_of = np.empty(E, np.int32)
    win_of = np.empty(E, np.int32)
    slot_of = np.empty(E, np.int32)
    for c, (wins, e0) in enumerate(cores):
        for w, (fn, wn, es, ne) in enumerate(wins):
            ids = perm[es:es + ne]
            core_of[ids] = c
            win_of[ids] = w
            slot_of[ids] = np.arange(ne)
    return core_of, win_of, slot_of


def _host_inputs(inputs):
    import ml_dtypes
    bf16 = ml_dtypes.bfloat16

    nl = np.asarray(inputs["node_left"], np.float32)
    nr = np.asarray(inputs["node_right"], np.float32)
    Wk = np.asarray(inputs["Wk"], np.float32)
    Wv = np.asarray(inputs["Wv"], np.float32)
    Wo = np.asarray(inputs["Wo"], np.float32)
    bo = np.asarray(inputs["bo"], np.float32)
    sl = np.asarray(inputs["segmentation_index_left"], np.int64)
    sr = np.asarray(inputs["segmentation_index_right"], np.int64)

    permL, sdL, coresL, wL = _prep_side(sl)
    permR, sdR, coresR, wR = _prep_side(sr)
    W = max(wL, wR)
    npad = W * BLK

    snL, nsL = _slot_maps(coresL, W)     # left-node slots
    snR, nsR = _slot_maps(coresR, W)     # right-node slots
    LcoreE, LwinE, LslotE = _edge_locs(permL, coresL)

    # ---- per-core L arrays ----
    ssL = sr[permL]          # src (right) node per L-sorted edge
    arrL = []
    for c, (wins, e0) in enumerate(coresL):
        sidx = np.zeros((W, WCAP), np.int64)      # src rows in R table
        drel = np.full((W, WCAP), DUMMY_REL, np.float32)
        for w, (fn, wn, es, ne) in enumerate(wins):
            sidx[w, :ne] = nsR[ssL[es:es + ne]]
            rel = (sdL[es:es + ne] - fn)
            drel[w, :ne] = rel.astype(np.float32)
        sidx16 = np.concatenate([_wrap_idx16(sidx[w]) for w in range(W)], 1)
        drelT = np.ascontiguousarray(
            drel.reshape(W, BPW, BLK).transpose(2, 0, 1).reshape(BLK, W * BPW))
        arrL.append(dict(sidx=sidx16, drel=drelT, colnode=snL[c]))

    # ---- per-core R arrays ----
    ssR = sl[permR]          # src (left) node per R-sorted edge
    arrR = []
    for c, (wins, e0) in enumerate(coresR):
        vidx = np.zeros((W, WCAP), np.int64)      # src rows in L V table
        eidx = np.zeros((W, WCAP), np.int64)      # escore row (64-col rows)
        ecol = np.zeros((W, WCAP), np.int64)
        emask_valid = np.zeros((W, WCAP), bool)
        drel = np.full((W, WCAP), DUMMY_REL, np.float32)
        for w, (fn, wn, es, ne) in enumerate(wins):
            ids = permR[es:es + ne]
            vidx[w, :ne] = nsL[ssR[es:es + ne]]
            # flat L escore position: core*(W*1024) + win*1024 + p*8 + b
            pL = LslotE[ids] % BLK
            bL = LslotE[ids] // BLK
            gpos = (LcoreE[ids].astype(np.int64) * W + LwinE[ids]) * WCAP \
                + pL * BPW + bL
            eidx[w, :ne] = gpos // 64
            ecol[w, :ne] = gpos % 64
            emask_valid[w, :ne] = True
            drel[w, :ne] = (sdR[es:es + ne] - fn).astype(np.float32)
        vidx16 = np.concatenate([_wrap_idx16(vidx[w]) for w in range(W)], 1)
        eidx16 = np.concatenate([_wrap_idx16(eidx[w]) for w in range(W)], 1)
        drelT = np.ascontiguousarray(
            drel.reshape(W, BPW, BLK).transpose(2, 0, 1).reshape(BLK, W * BPW))
        emask = np.zeros((BLK, W * BPW * 64), np.float32)
        wv, iv = np.nonzero(emask_valid)
        pv, bv = iv % BLK, iv // BLK
        emask[pv, (wv * BPW + bv) * 64 + ecol[wv, iv]] = 1.0
        arrR.append(dict(vidx=vidx16, eidx=eidx16, drel=drelT,
                         emask=emask.astype(bf16), colnode=snR[c]))

    # ---- shared constants ----
    Wkv = np.concatenate([Wk, Wv], 0)               # [1024, 512]
    WkvT = Wkv.T                                    # [512, 1024]
    wkvT_arr = np.zeros((128, 4 * 1024), np.float32)
    for cc in range(4):
        wkvT_arr[:, cc * 1024:(cc + 1) * 1024] = \
            WkvT[cc * 128:(cc + 1) * 128, :]
    # wkT: lhsT tiles for the transposed K GEMM
    wkT_arr = np.zeros((128, 16 * 128), np.float32)
    for i in range(4):
        for o in range(4):
            wkT_arr[:, (i * 4 + o) * 128:(i * 4 + o + 1) * 128] = \
                Wk[o * 128:(o + 1) * 128, i * 128:(i + 1) * 128].T
    woT_arr = np.zeros((128, 4 * 512), np.float32)
    for cc in range(4):
        for oc in range(4):
            woT_arr[:, cc * 512 + oc * 128: cc * 512 + (oc + 1) * 128] = \
                Wo[oc * 128:(oc + 1) * 128, cc * 128:(cc + 1) * 128].T
    bo_arr = bo.reshape(4, 128).T.copy()            # [128, 4]
    iota_arr = np.broadcast_to(
        np.arange(128, dtype=np.float32)[None, :], (128, 128))
    ident_arr = np.eye(128, dtype=np.float32)

    def shardT(feat, slot_node_c):
        sh = np.zeros((C, npad), np.float32)
        m = slot_node_c >= 0
        sh[:, m] = feat[slot_node_c[m]].T
        return np.ascontiguousarray(sh).astype(bf16)

    in_maps = []
    for c in range(NCORES):
        in_maps.append({
            "nT_L": shardT(nl, snL[c]),
            "nT_R": shardT(nr, snR[c]),
            "wkvT": wkvT_arr.astype(bf16),
            "wkT": wkT_arr.astype(bf16),
            "woT": woT_arr.astype(bf16),
            "bo": bo_arr,
            "iota": np.ascontiguousarray(iota_arr).astype(bf16),
            "ident": np.ascontiguousarray(ident_arr).astype(bf16),
            "sidx_L": arrL[c]["sidx"],
            "drel_L": arrL[c]["drel"],
            "vidx_R": arrR[c]["vidx"],
            "eidx_R": arrR[c]["eidx"],
            "drel_R": arrR[c]["drel"],
            "emask_R": arrR[c]["emask"],
        })
    return in_maps, arrL, arrR, W


def _build_program(W):
    import concourse.bacc as bacc
    import concourse.tile as tile
    from concourse import mybir

    dt = mybir.dt
    f32, bf16, i16 = dt.float32, dt.bfloat16, dt.int16
    AF = mybir.ActivationFunctionType
    OP = mybir.AluOpType
    npad = W * BLK
    ntiles = W

    nc = bacc.Bacc("TRN2", target_bir_lowering=False, debug=False,
                   enable_asserts=True, num_devices=NCORES,
                   num_swdge_queues=2)

    # ---- I/O ----
    nT_in = {s: nc.dram_tensor(f"nT_{s}", [C, npad], bf16,
                               kind="ExternalInput").ap() for s in "LR"}
    wkvT = nc.dram_tensor("wkvT", [128, 4 * 1024], bf16,
                          kind="ExternalInput").ap()
    wkT_in = nc.dram_tensor("wkT", [128, 16 * 128], bf16,
                            kind="ExternalInput").ap()
    woT = nc.dram_tensor("woT", [128, 4 * 512], bf16,
                         kind="ExternalInput").ap()
    bo_in = nc.dram_tensor("bo", [128, 4], f32, kind="ExternalInput").ap()
    iota_in = nc.dram_tensor("iota", [128, 128], bf16,
                             kind="ExternalInput").ap()
    ident_in = nc.dram_tensor("ident", [128, 128], bf16,
                              kind="ExternalInput").ap()
    sidx_in = nc.dram_tensor("sidx_L", [128, W * 64], i16,
                             kind="ExternalInput").ap()
    drelL_in = nc.dram_tensor("drel_L", [128, W * BPW], f32,
                              kind="ExternalInput").ap()
    vidx_in = nc.dram_tensor("vidx_R", [128, W * 64], i16,
                             kind="ExternalInput").ap()
    eidx_in = nc.dram_tensor("eidx_R", [128, W * 64], i16,
                             kind="ExternalInput").ap()
    drelR_in = nc.dram_tensor("drel_R", [128, W * BPW], f32,
                              kind="ExternalInput").ap()
    emask_in = nc.dram_tensor("emask_R", [128, W * BPW * 64], bf16,
                              kind="ExternalInput").ap()
    hT_out = {s: nc.dram_tensor(f"hT_{s}", [C, npad], bf16,
                                kind="ExternalOutput").ap() for s in "LR"}

    # ---- internal DRAM ----
    tkv_shR = nc.dram_tensor("tkv_shR", [npad, 2 * C], bf16).ap()
    tv_shL = nc.dram_tensor("tv_shL", [npad, C], bf16).ap()
    esc_sh = nc.dram_tensor("esc_sh", [W, 128, BPW], f32).ap()
    tkv_R = nc.dram_tensor("tkv_R", [NCORES * npad, 2 * C], bf16,
                           addr_space="Shared").ap()
    tv_L = nc.dram_tensor("tv_L", [NCORES * npad, C], bf16,
                          addr_space="Shared").ap()
    esc_full = nc.dram_tensor("esc_full", [NCORES * W * 16, 64], f32,
                              addr_space="Shared").ap()

    with tile.TileContext(nc) as tc:
        with tc.tile_pool(name="const", bufs=1) as cpool:
            # early constants (phase A + L loop)
            wkvT_sb = cpool.tile([128, 4 * 1024], bf16)
            nc.sync.dma_start(wkvT_sb[:], wkvT[:, :])
            wkT_sb = cpool.tile([128, 16 * 128], bf16)
            nc.sync.dma_start(wkT_sb[:], wkT_in[:, :])
            sidx_sb = cpool.tile([128, W * 64], i16)
            nc.sync.dma_start(sidx_sb[:], sidx_in[:, :])
            drelL_sb = cpool.tile([128, W * BPW], f32)
            nc.sync.dma_start(drelL_sb[:], drelL_in[:, :])
            ident_sb = cpool.tile([128, 128], bf16)
            nc.sync.dma_start(ident_sb[:], ident_in[:, :])
            woT_sb = cpool.tile([128, 4 * 512], bf16)
            nc.sync.dma_start(woT_sb[:], woT[:, :])
            bo_sb = cpool.tile([128, 4], f32)
            nc.sync.dma_start(bo_sb[:], bo_in[:, :])
            iota_sb = cpool.tile([128, 128], bf16)
            nc.sync.dma_start(iota_sb[:], iota_in[:, :])
            ones_col = cpool.tile([128, 1], bf16)
            nc.vector.memset(ones_col[:], 1.0)
            ones_row = cpool.tile([1, 128], bf16)
            nc.vector.memset(ones_row[:], 1.0)
            # late constants (R loop only; loaded after phase A issues)
            vidx_sb = cpool.tile([128, W * 64], i16)
            eidx_sb = cpool.tile([128, W * 64], i16)
            drelR_sb = cpool.tile([128, W * BPW], f32)
            emask_sb = cpool.tile([128, W * BPW * 64], bf16)
            hacc = cpool.tile([128, 4 * npad], bf16)
            klT_sb = cpool.tile([128, 4, npad], bf16)   # left K transposed

            # ---- phase A ----
            with (
                tc.tile_pool(name="feat", bufs=1) as fpool,
                tc.tile_pool(name="gemm_sb", bufs=3) as gsb,
                tc.tile_pool(name="psum_gemm", bufs=2, space="PSUM") as pg,
            ):
                featR = []
                for cc in range(4):
                    t = fpool.tile([128, npad], bf16, tag=f"featR{cc}")
                    nc.sync.dma_start(
                        t[:], nT_in["R"][cc * 128:(cc + 1) * 128, :])
                    featR.append(t)
                # R side K|V fused GEMM -> tkv_shR -> AG#1
                for ti in range(ntiles):
                    sb = gsb.tile([128, 1024], bf16)
                    for half in range(2):
                        ps = pg.tile([128, 512], f32)
                        for cc in range(4):
                            nc.tensor.matmul(
                                ps[:],
                                lhsT=featR[cc][:, ti * 128:(ti + 1) * 128],
                                rhs=wkvT_sb[:, cc * 1024 + half * 512:
                                            cc * 1024 + half * 512 + 512],
                                start=(cc == 0), stop=(cc == 3))
                        if half == 0:
                            nc.vector.tensor_copy(
                                sb[:, 0:512], ps[:])
                        else:
                            nc.scalar.copy(
                                sb[:, 512:1024], ps[:])
                    nc.sync.dma_start(
                        tkv_shR[ti * 128:(ti + 1) * 128, :], sb[:])
                if not SKIP_AG:
                    nc.gpsimd.collective_compute(
                        "AllGather", mybir.AluOpType.bypass,
                        replica_groups=[list(range(NCORES))],
                        ins=[tkv_shR], outs=[tkv_R])
                else:
                    # timing build: sliver copy keeps the dependency edge
                    nc.sync.dma_start(tkv_R[0:128, :], tkv_shR[0:128, :])

                featL = []
                for cc in range(4):
                    t = fpool.tile([128, npad], bf16, tag=f"featL{cc}")
                    nc.sync.dma_start(
                        t[:], nT_in["L"][cc * 128:(cc + 1) * 128, :])
                    featL.append(t)
                # L side transposed-K GEMM -> klT_sb (stays in SBUF)
                for o in range(4):
                    for nb in range(math.ceil(npad / 512)):
                        n0 = nb * 512
                        n1 = min(npad, n0 + 512)
                        ps = pg.tile([128, 512], f32)
                        for i in range(4):
                            nc.tensor.matmul(
                                ps[:, 0:n1 - n0],
                                lhsT=wkT_sb[:, (i * 4 + o) * 128:
                                            (i * 4 + o + 1) * 128],
                                rhs=featL[i][:, n0:n1],
                                start=(i == 0), stop=(i == 3))
                        nc.scalar.copy(
                            klT_sb[:, o, n0:n1], ps[:, 0:n1 - n0])
                # L side V GEMM -> tv_shL -> AG#2
                for ti in range(ntiles):
                    sb = gsb.tile([128, 512], bf16, tag="sbv")
                    ps = pg.tile([128, 512], f32)
                    for cc in range(4):
                        nc.tensor.matmul(
                            ps[:],
                            lhsT=featL[cc][:, ti * 128:(ti + 1) * 128],
                            rhs=wkvT_sb[:, cc * 1024 + 512:
                                        cc * 1024 + 1024],
                            start=(cc == 0), stop=(cc == 3))
                    nc.vector.tensor_copy(sb[:], ps[:])
                    nc.sync.dma_start(
                        tv_shL[ti * 128:(ti + 1) * 128, :], sb[:])
                if not SKIP_AG:
                    nc.gpsimd.collective_compute(
                        "AllGather", mybir.AluOpType.bypass,
                        replica_groups=[list(range(NCORES))],
                        ins=[tv_shL], outs=[tv_L])
                else:
                    nc.sync.dma_start(tv_L[0:128, :], tv_shL[0:128, :])

            # late const loads (R loop)
            nc.sync.dma_start(vidx_sb[:], vidx_in[:, :])
            nc.sync.dma_start(eidx_sb[:], eidx_in[:, :])
            nc.sync.dma_start(drelR_sb[:], drelR_in[:, :])
            nc.sync.dma_start(emask_sb[:], emask_in[:, :])

            nidx_reg = nc.gpsimd.to_reg(WCAP)
            nidx_reg2 = nc.gpsimd.to_reg(WCAP // 2)

            with (
                tc.tile_pool(name="gath", bufs=3) as gpool,
                tc.tile_pool(name="aux2", bufs=3) as kdpool,
                tc.tile_pool(name="blk", bufs=4) as sp,
                tc.tile_pool(name="ebuf", bufs=4) as ebpool,
                tc.tile_pool(name="etbuf", bufs=2 * BPW) as ohpool,
                tc.tile_pool(name="tail", bufs=3) as tp,
                tc.tile_pool(name="pMT", bufs=2, space="PSUM") as pMT,
                tc.tile_pool(name="pmsg", bufs=2, space="PSUM") as pmsg,
                tc.tile_pool(name="paux", bufs=1, space="PSUM") as paux,
                tc.tile_pool(name="ph", bufs=1, space="PSUM") as ph,
            ):
                # ---- phase C-L: matmul scores + left messages ----
                for w in ([] if (SKIP_C or SKIP_L) else range(W)):
                    # transposed K gather split in two: the worker's ucode
                    # fails above 512 indices per transpose gather
                    ktgs = []
                    for h in range(2):
                        kt = gpool.tile([128, 4, WCAP // 2], bf16,
                                        tag=f"ktg{h}")
                        nc.gpsimd.dma_gather(
                            kt[:], tkv_R[:, 0:C],
                            sidx_sb[:, w * 64 + h * 32: w * 64 + h * 32 + 32],
                            WCAP // 2, nidx_reg2, C, elem_step=2 * C,
                            transpose=True)
                        ktgs.append(kt)
                    vg = gpool.tile([128, BPW, C], bf16, tag="vg")
                    nc.gpsimd.dma_gather(
                        vg[:], tkv_R[:, C:2 * C],
                        sidx_sb[:, w * 64:(w + 1) * 64],
                        WCAP, nidx_reg, C, elem_step=2 * C, queue_num=1)
                    ehs = sp.tile([128, BPW], f32, tag="ehs")
                    ets = []
                    msgT_ps = pmsg.tile([128, 512], f32)
                    z_ps = paux.tile([128, 128], f32, tag="aux")
                    for b in range(BPW):
                        M_ps = pMT.tile([128, 128], f32, tag="M")
                        kth = ktgs[b // 4]
                        bh = b % 4
                        for j in range(4):
                            nc.tensor.matmul(
                                M_ps[:],
                                lhsT=klT_sb[:, j, w * 128:(w + 1) * 128],
                                rhs=kth[:, j, bh * 128:(bh + 1) * 128],
                                start=(j == 0), stop=(j == 3))
                        Mc = ebpool.tile([128, 128], bf16, tag="Mc")
                        nc.scalar.mul(Mc[:], M_ps[:], 1.0 / TEMP)
                        T_ps = pMT.tile([128, 128], bf16, tag="T")
                        nc.tensor.transpose(T_ps[:], Mc[:], ident_sb[:])
                        eb = ebpool.tile([128, 128], bf16, tag="eb")
                        nc.scalar.activation(eb[:], T_ps[:], AF.Exp)
                        q = ebpool.tile([128, 128], bf16, tag="q")
                        nc.vector.tensor_scalar(
                            q[:], iota_sb[:],
                            drelL_sb[:, w * BPW + b: w * BPW + b + 1],
                            1.0, op0=OP.is_equal, op1=OP.mult)
                        et = ohpool.tile([128, 128], bf16, tag="et")
                        nc.vector.tensor_tensor(et[:], eb[:], q[:],
                                                op=OP.mult)
                        # eh per edge = row-sum of E^T (raw, pre-normalize)
                        nc.vector.tensor_reduce(
                            ehs[:, b:b + 1], et[:],
                            axis=mybir.AxisListType.X, op=OP.add)
                        ets.append(et)
                    # escore shard write: esc[w, p, b] = eh[p, b]
                    nc.sync.dma_start(esc_sh[w, :, :], ehs[:])

                    for cc in range(4):
                        for b in range(BPW):
                            nc.tensor.matmul(
                                msgT_ps[:, cc * 128:(cc + 1) * 128],
                                lhsT=vg[:, b, cc * 128:(cc + 1) * 128],
                                rhs=ets[b][:],
                                start=(b == 0), stop=(b == BPW - 1))
                    for b in range(BPW):
                        nc.tensor.matmul(
                            z_ps[0:1, :], lhsT=ones_col[:], rhs=ets[b][:],
                            start=(b == 0), stop=(b == BPW - 1))

                    _window_tail(nc, tc, mybir, w, msgT_ps, z_ps,
                                 tp, paux, ph, ones_row, woT_sb, bo_sb,
                                 hacc, W)
                for oc in ([] if (SKIP_C or SKIP_L) else range(4)):
                    nc.sync.dma_start(
                        hT_out["L"][oc * 128:(oc + 1) * 128, :],
                        hacc[:, oc * npad:(oc + 1) * npad])

                # ---- AG#3: escore ----
                if not SKIP_C:
                    if not SKIP_AG:
                        nc.gpsimd.collective_compute(
                            "AllGather", mybir.AluOpType.bypass,
                            replica_groups=[list(range(NCORES))],
                            ins=[esc_sh], outs=[esc_full])
                    else:
                        nc.sync.dma_start(esc_full[0:2, :],
                                          esc_sh[0, 0:16, :])

                # ---- phase C-R: right messages from shared scores ----
                for w in ([] if (SKIP_C or SKIP_R) else range(W)):
                    v = gpool.tile([128, BPW, C], bf16, tag="v")
                    nc.gpsimd.dma_gather(
                        v[:], tv_L[:, :], vidx_sb[:, w * 64:(w + 1) * 64],
                        WCAP, nidx_reg, C)
                    eg = kdpool.tile([128, BPW, 64], f32, tag="eg")
                    nc.gpsimd.dma_gather(
                        eg[:], esc_full[:, :], eidx_sb[:, w * 64:(w + 1) * 64],
                        WCAP, nidx_reg, 64, queue_num=1)

                    # select each edge's eh via the host-shipped one-hot mask
                    egm = sp.tile([128, BPW, 64], bf16, tag="egm")
                    nc.gpsimd.tensor_tensor(
                        egm[:], eg[:, :, :],
                        emask_sb[:, w * BPW * 64:(w + 1) * BPW * 64],
                        op=OP.mult)
                    ehR = sp.tile([128, BPW], f32, tag="ehR")
                    nc.vector.tensor_reduce(
                        ehR[:], egm[:, :, :],
                        axis=mybir.AxisListType.X, op=OP.add)

                    ohs = []
                    for b in range(BPW):
                        oh = ohpool.tile([128, 128], bf16, tag="et")
                        nc.vector.tensor_scalar(
                            oh[:], iota_sb[:],
                            drelR_sb[:, w * BPW + b: w * BPW + b + 1],
                            ehR[:, b:b + 1], op0=OP.is_equal, op1=OP.mult)
                        ohs.append(oh)

                    msgT_ps = pmsg.tile([128, 512], f32)
                    z_ps = paux.tile([128, 128], f32, tag="aux")
                    for cc in range(4):
                        for b in range(BPW):
                            nc.tensor.matmul(
                                msgT_ps[:, cc * 128:(cc + 1) * 128],
                                lhsT=v[:, b, cc * 128:(cc + 1) * 128],
                                rhs=ohs[b][:],
                                start=(b == 0), stop=(b == BPW - 1))
                    for b in range(BPW):
                        nc.tensor.matmul(
                            z_ps[0:1, :], lhsT=ones_col[:], rhs=ohs[b][:],
                            start=(b == 0), stop=(b == BPW - 1))

                    _window_tail(nc, tc, mybir, w, msgT_ps, z_ps,
                                 tp, paux, ph, ones_row, woT_sb, bo_sb,
                                 hacc, W)
                for oc in ([] if (SKIP_C or SKIP_R) else range(4)):
                    nc.sync.dma_start(
                        hT_out["R"][oc * 128:(oc + 1) * 128, :],
                        hacc[:, oc * npad:(oc + 1) * npad])
    nc.compile()
    return nc


def _window_tail(nc, tc, mybir, w, msgT_ps, z_ps, tp, paux, ph,
                 ones_row, woT_sb, bo_sb, hacc, W):
    """z -> 1/z broadcast, msgT normalize, Wo GEMM, bias+LeakyReLU."""
    f32, bf16 = mybir.dt.float32, mybir.dt.bfloat16
    AF = mybir.ActivationFunctionType
    OP = mybir.AluOpType
    npad = W * 128

    zm = tp.tile([1, 128], f32, tag="zm")
    nc.vector.tensor_scalar_max(zm[:], z_ps[0:1, :], 1e-30)
    zr = tp.tile([1, 128], f32, tag="zr")
    nc.vector.reciprocal(zr[:], zm[:])
    zrb = tp.tile([1, 128], bf16, tag="zrb")
    nc.vector.tensor_copy(zrb[:], zr[:])
    zbc_ps = paux.tile([128, 128], f32, tag="aux")
    nc.tensor.matmul(zbc_ps[:], lhsT=ones_row[:], rhs=zrb[:],
                     start=True, stop=True)
    zbc = tp.tile([128, 128], f32, tag="zbc")
    nc.scalar.copy(zbc[:], zbc_ps[:])
    msgT_sb = tp.tile([128, 512], bf16, tag="msgT")
    for nch in range(4):
        nc.vector.tensor_tensor(
            msgT_sb[:, nch * 128:(nch + 1) * 128],
            msgT_ps[:, nch * 128:(nch + 1) * 128],
            zbc[:], op=OP.mult)
    hT_ps = ph.tile([128, 512], f32)
    for oc in range(4):
        for cc in range(4):
            nc.tensor.matmul(
                hT_ps[:, oc * 128:(oc + 1) * 128],
                lhsT=woT_sb[:, cc * 512 + oc * 128:
                            cc * 512 + oc * 128 + 128],
                rhs=msgT_sb[:, cc * 128:(cc + 1) * 128],
                start=(cc == 0), stop=(cc == 3))
    for oc in range(4):
        x = hacc[:, oc * npad + w * 128: oc * npad + (w + 1) * 128]
        nc.scalar.activation(
            x, hT_ps[:, oc * 128:(oc + 1) * 128],
            AF.Identity, bias=bo_sb[:, oc:oc + 1])
        x2 = tp.tile([128, 128], bf16, tag="x2")
        nc.vector.tensor_scalar_mul(x2[:], x, NEG)
        nc.vector.tensor_tensor(x, x, x2[:], op=OP.max)


def _assemble(results, arrs, key):
    out = np.zeros((N, C), np.float32)
    for c in range(NCORES):
        hT = np.asarray(results[c][key], np.float32)
        cn = arrs[c]["colnode"]
        m = cn >= 0
        out[cn[m]] = hT[:, m].T
    return out


_RUN_KWARGS = {}


def kernel(**inputs):
    from concourse.bass_utils import run_bass_kernel_spmd

    in_maps, arrL, arrR, W = _host_inputs(inputs)
    nc = _build_program(W)
    res = run_bass_kernel_spmd(nc, in_maps, core_ids=list(range(NCORES)),
                               **_RUN_KWARGS)
    out_l = _assemble(res.results, arrL, "hT_L")
    out_r = _assemble(res.results, arrR, "hT_R")
    kernel.last_results = res
    kernel.last_nc = nc
    kernel.last_W = W
    return (out_l, out_r)


# revision 30
# speedup vs baseline: 1.4194x; 1.0201x over previous
"""Trainium2 Bass kernel for nn_MultiHeadAttention_73589969649754
(gnn_message_passing / graph cross-attention).

v3 strategy (score sharing + matmul scoring):
  - Edges sorted by destination per side; core c owns nodes
    [c*2500, (c+1)*2500) on both sides.  Windows of <=128 consecutive
    nodes / <=1024 edge slots.  The HOST permutes each core's node
    columns so window w occupies slots [w*128, (w+1)*128) - all program
    offsets are SPMD-uniform and tables are slot-ordered.
  - Phase A: R side emits a fused K|V table (AllGather #1, 5.24MB/rank);
    L side computes K TRANSPOSED ([channel, slot], kept in SBUF - no
    DRAM round trip) and a V table (AllGather #2, 2.62MB/rank).
  - L pass per window: one TRANSPOSED dma_gather pulls Kr[src] in
    [channel, edge] orientation and a normal gather pulls Vr[src].
    Scores come from PE matmuls M[slot, edge] = KlT_win^T @ KrT_gath
    (no per-edge dot product on DVE/ACT), then exp on ACT, a host-shipped
    one-hot mask zeroes off-segment entries, and a PE transpose yields
    E^T[edge, slot] - which IS the eh-scaled one-hot the segment-sum
    matmuls consume.  eh per edge (row-sum of E^T) is written to a score
    shard (AllGather #3, 80KB/rank).  z via ones-matmul; messages are
    normalized after the fact by 1/z (column broadcast); Wo GEMM;
    bias+LeakyReLU.
  - R pass: gathers only Vl[src] rows (1KB/edge) plus 256B score-table
    rows; a host-shipped one-hot mask selects each edge's eh (softmax
    numerators are shared between the two sides).  No K gather, no dot
    product, no exp on the R side.
  - Outputs stored bf16 in [channel, slot] layout; host reassembles.
"""

import math

import numpy as np

N = 20000
E = 160000
C = 512
NCORES = 8
TEMP = float(np.sqrt(C))
NEG = 0.01
NPC = N // NCORES            # 2500 nodes per core per side
BLK = 128                    # edges per block
BPW = 8                      # blocks per window
WCAP = BPW * BLK             # 1024 edge slots per window
DUMMY_REL = 999.0
SKIP_AG = False
SKIP_C = False
SKIP_L = False       # debug: skip the L window loop
SKIP_R = False       # debug: skip the R window loop

# AllGather wall-time charge (ns): measured-table upper bound is ~70us for
# a 5.24MB/rank 8-way intra-chip AllGather; scale by bytes with a 20us
# latency floor.  Serial charges: AG#1 tkv_R 5.24MB -> 70us (gates the L
# loop) and AG#3 escore 80KB -> 20us (floor; gates the R score gathers).
# AG#2 (tv_L, 2.62MB -> ~35us) runs on the collective cores/links right
# after AG#1 (~105us done) and is only consumed by the R loop, which
# starts after the ~175us L loop - fully hidden, so not charged.
AG_CHARGE_NS = 70e3 + 20e3


def _prep_side(seg_dst):
    """Sort edges by dst; node-aligned core ranges; pack windows."""
    seg_dst = np.asarray(seg_dst, np.int64)
    perm = np.argsort(seg_dst, kind="stable")
    sd = seg_dst[perm]
    deg = np.bincount(sd, minlength=N)
    edge_b = [int(np.searchsorted(sd, c * NPC, "left"))
              for c in range(NCORES)] + [E]

    cores = []
    max_w = 0
    for c in range(NCORES):
        n0, n1 = c * NPC, (c + 1) * NPC
        e0 = edge_b[c]
        wins = []
        n, e = n0, e0
        while n < n1:
            wn = we = 0
            while n + wn < n1 and wn < BLK and we + deg[n + wn] <= WCAP:
                we += deg[n + wn]
                wn += 1
            assert wn > 0, "node degree exceeds window capacity"
            wins.append((n, wn, e, we))
            n += wn
            e += we
        assert e == edge_b[c + 1]
        cores.append((wins, e0))
        max_w = max(max_w, len(wins))
    return perm, sd, cores, max_w


def _wrap_idx16(idx_flat):
    """[n] -> [128, n//16] int16, i at [i%16, i//16], replicated x8."""
    n = idx_flat.shape[0]
    a = idx_flat.reshape(n // 16, 16).T.astype(np.int16)
    return np.ascontiguousarray(np.tile(a, (8, 1)))


def _slot_maps(cores, W):
    """Slot-order the nodes: window w of core c occupies slots
    [w*128, (w+1)*128).  Returns (slot_node [NCORES, W*128] node-or--1,
    node_slot [N] global slot = core*W*128 + slot)."""
    npad = W * BLK
    slot_node = np.full((NCORES, npad), -1, np.int64)
    node_slot = np.full(N, -1, np.int64)
    for c, (wins, e0) in enumerate(cores):
        for w, (fn, wn, es, ne) in enumerate(wins):
            sl = np.arange(wn)
            slot_node[c, w * BLK + sl] = fn + sl
            node_slot[fn + sl] = c * npad + w * BLK + sl
    assert (node_slot >= 0).all()
    return slot_node, node_slot


def _edge_locs(perm, cores):
    """Per original edge id: (core, window, slot-in-window) on this side."""
    core_of = np.empty(E, np.int32)
    win_of = np.empty(E, np.int32)
    slot_of = np.empty(E, np.int32)
    for c, (wins, e0) in enumerate(cores):
        for w, (fn, wn, es, ne) in enumerate(wins):
            ids = perm[es:es + ne]
            core_of[ids] = c
            win_of[ids] = w
            slot_of[ids] = np.arange(ne)
    return core_of, win_of, slot_of


def _host_inputs(inputs):
    import ml_dtypes
    bf16 = ml_dtypes.bfloat16

    nl = np.asarray(inputs["node_left"], np.float32)
    nr = np.asarray(inputs["node_right"], np.float32)
    Wk = np.asarray(inputs["Wk"], np.float32)
    Wv = np.asarray(inputs["Wv"], np.float32)
    Wo = np.asarray(inputs["Wo"], np.float32)
    bo = np.asarray(inputs["bo"], np.float32)
    sl = np.asarray(inputs["segmentation_index_left"], np.int64)
    sr = np.asarray(inputs["segmentation_index_right"], np.int64)

    permL, sdL, coresL, wL = _prep_side(sl)
    permR, sdR, coresR, wR = _prep_side(sr)
    W = max(wL, wR)
    npad = W * BLK

    snL, nsL = _slot_maps(coresL, W)     # left-node slots
    snR, nsR = _slot_maps(coresR, W)     # right-node slots
    LcoreE, LwinE, LslotE = _edge_locs(permL, coresL)

    # ---- per-core L arrays ----
    ssL = sr[permL]          # src (right) node per L-sorted edge
    arrL = []
    for c, (wins, e0) in enumerate(coresL):
        sidx = np.zeros((W, WCAP), np.int64)      # src rows in R table
        drel = np.full((W, WCAP), DUMMY_REL, np.float32)
        qtm = np.zeros((BLK, W * WCAP), np.float32)
        for w, (fn, wn, es, ne) in enumerate(wins):
            sidx[w, :ne] = nsR[ssL[es:es + ne]]
            rel = (sdL[es:es + ne] - fn)
            drel[w, :ne] = rel.astype(np.float32)
            # QT mask: [dst-rel, (w*8+b)*128 + p] = 1 for edge slot i=b*128+p
            i = np.arange(ne)
            qtm[rel, w * WCAP + (i // BLK) * BLK + (i % BLK)] = 1.0
        sidx16 = np.concatenate([_wrap_idx16(sidx[w]) for w in range(W)], 1)
        drelT = np.ascontiguousarray(
            drel.reshape(W, BPW, BLK).transpose(2, 0, 1).reshape(BLK, W * BPW))
        arrL.append(dict(sidx=sidx16, drel=drelT, qtm=qtm.astype(bf16),
                         colnode=snL[c]))

    # ---- per-core R arrays ----
    ssR = sl[permR]          # src (left) node per R-sorted edge
    arrR = []
    for c, (wins, e0) in enumerate(coresR):
        vidx = np.zeros((W, WCAP), np.int64)      # src rows in L V table
        eidx = np.zeros((W, WCAP), np.int64)      # escore row (64-col rows)
        ecol = np.zeros((W, WCAP), np.int64)
        emask_valid = np.zeros((W, WCAP), bool)
        drel = np.full((W, WCAP), DUMMY_REL, np.float32)
        for w, (fn, wn, es, ne) in enumerate(wins):
            ids = permR[es:es + ne]
            vidx[w, :ne] = nsL[ssR[es:es + ne]]
            # flat L escore position: core*(W*1024) + win*1024 + p*8 + b
            pL = LslotE[ids] % BLK
            bL = LslotE[ids] // BLK
            gpos = (LcoreE[ids].astype(np.int64) * W + LwinE[ids]) * WCAP \
                + pL * BPW + bL
            eidx[w, :ne] = gpos // 64
            ecol[w, :ne] = gpos % 64
            emask_valid[w, :ne] = True
            drel[w, :ne] = (sdR[es:es + ne] - fn).astype(np.float32)
        vidx16 = np.concatenate([_wrap_idx16(vidx[w]) for w in range(W)], 1)
        eidx16 = np.concatenate([_wrap_idx16(eidx[w]) for w in range(W)], 1)
        drelT = np.ascontiguousarray(
            drel.reshape(W, BPW, BLK).transpose(2, 0, 1).reshape(BLK, W * BPW))
        emask = np.zeros((BLK, W * BPW * 64), np.float32)
        wv, iv = np.nonzero(emask_valid)
        pv, bv = iv % BLK, iv // BLK
        emask[pv, (wv * BPW + bv) * 64 + ecol[wv, iv]] = 1.0
        arrR.append(dict(vidx=vidx16, eidx=eidx16, drel=drelT,
                         emask=emask.astype(bf16), colnode=snR[c]))

    # ---- shared constants ----
    Wkv = np.concatenate([Wk, Wv], 0)               # [1024, 512]
    WkvT = Wkv.T                                    # [512, 1024]
    wkvT_arr = np.zeros((128, 4 * 1024), np.float32)
    for cc in range(4):
        wkvT_arr[:, cc * 1024:(cc + 1) * 1024] = \
            WkvT[cc * 128:(cc + 1) * 128, :]
    # wkT: lhsT tiles for the transposed K GEMM
    wkT_arr = np.zeros((128, 16 * 128), np.float32)
    for i in range(4):
        for o in range(4):
            wkT_arr[:, (i * 4 + o) * 128:(i * 4 + o + 1) * 128] = \
                Wk[o * 128:(o + 1) * 128, i * 128:(i + 1) * 128].T
    woT_arr = np.zeros((128, 4 * 512), np.float32)
    for cc in range(4):
        for oc in range(4):
            woT_arr[:, cc * 512 + oc * 128: cc * 512 + (oc + 1) * 128] = \
                Wo[oc * 128:(oc + 1) * 128, cc * 128:(cc + 1) * 128].T
    bo_arr = bo.reshape(4, 128).T.copy()            # [128, 4]
    iota_arr = np.broadcast_to(
        np.arange(128, dtype=np.float32)[None, :], (128, 128))
    ident_arr = np.eye(128, dtype=np.float32)

    def shardT(feat, slot_node_c):
        sh = np.zeros((C, npad), np.float32)
        m = slot_node_c >= 0
        sh[:, m] = feat[slot_node_c[m]].T
        return np.ascontiguousarray(sh).astype(bf16)

    in_maps = []
    for c in range(NCORES):
        in_maps.append({
            "nT_L": shardT(nl, snL[c]),
            "nT_R": shardT(nr, snR[c]),
            "wkvT": wkvT_arr.astype(bf16),
            "wkT": wkT_arr.astype(bf16),
            "woT": woT_arr.astype(bf16),
            "bo": bo_arr,
            "iota": np.ascontiguousarray(iota_arr).astype(bf16),
            "ident": np.ascontiguousarray(ident_arr).astype(bf16),
            "sidx_L": arrL[c]["sidx"],
            "drel_L": arrL[c]["drel"],
            "qtm_L": arrL[c]["qtm"],
            "vidx_R": arrR[c]["vidx"],
            "eidx_R": arrR[c]["eidx"],
            "drel_R": arrR[c]["drel"],
            "emask_R": arrR[c]["emask"],
        })
    return in_maps, arrL, arrR, W


def _build_program(W):
    import concourse.bacc as bacc
    import concourse.tile as tile
    from concourse import mybir

    dt = mybir.dt
    f32, bf16, i16 = dt.float32, dt.bfloat16, dt.int16
    AF = mybir.ActivationFunctionType
    OP = mybir.AluOpType
    npad = W * BLK
    ntiles = W

    nc = bacc.Bacc("TRN2", target_bir_lowering=False, debug=False,
                   enable_asserts=True, num_devices=NCORES,
                   num_swdge_queues=2)

    # ---- I/O ----
    nT_in = {s: nc.dram_tensor(f"nT_{s}", [C, npad], bf16,
                               kind="ExternalInput").ap() for s in "LR"}
    wkvT = nc.dram_tensor("wkvT", [128, 4 * 1024], bf16,
                          kind="ExternalInput").ap()
    wkT_in = nc.dram_tensor("wkT", [128, 16 * 128], bf16,
                            kind="ExternalInput").ap()
    woT = nc.dram_tensor("woT", [128, 4 * 512], bf16,
                         kind="ExternalInput").ap()
    bo_in = nc.dram_tensor("bo", [128, 4], f32, kind="ExternalInput").ap()
    iota_in = nc.dram_tensor("iota", [128, 128], bf16,
                             kind="ExternalInput").ap()
    ident_in = nc.dram_tensor("ident", [128, 128], bf16,
                              kind="ExternalInput").ap()
    sidx_in = nc.dram_tensor("sidx_L", [128, W * 64], i16,
                             kind="ExternalInput").ap()
    drelL_in = nc.dram_tensor("drel_L", [128, W * BPW], f32,
                              kind="ExternalInput").ap()
    qtm_in = nc.dram_tensor("qtm_L", [128, W * WCAP], bf16,
                            kind="ExternalInput").ap()
    vidx_in = nc.dram_tensor("vidx_R", [128, W * 64], i16,
                             kind="ExternalInput").ap()
    eidx_in = nc.dram_tensor("eidx_R", [128, W * 64], i16,
                             kind="ExternalInput").ap()
    drelR_in = nc.dram_tensor("drel_R", [128, W * BPW], f32,
                              kind="ExternalInput").ap()
    emask_in = nc.dram_tensor("emask_R", [128, W * BPW * 64], bf16,
                              kind="ExternalInput").ap()
    hT_out = {s: nc.dram_tensor(f"hT_{s}", [C, npad], bf16,
                                kind="ExternalOutput").ap() for s in "LR"}

    # ---- internal DRAM ----
    tkv_shR = nc.dram_tensor("tkv_shR", [npad, 2 * C], bf16).ap()
    tv_shL = nc.dram_tensor("tv_shL", [npad, C], bf16).ap()
    esc_sh = nc.dram_tensor("esc_sh", [W, 128, BPW], f32).ap()
    tkv_R = nc.dram_tensor("tkv_R", [NCORES * npad, 2 * C], bf16,
                           addr_space="Shared").ap()
    tv_L = nc.dram_tensor("tv_L", [NCORES * npad, C], bf16,
                          addr_space="Shared").ap()
    esc_full = nc.dram_tensor("esc_full", [NCORES * W * 16, 64], f32,
                              addr_space="Shared").ap()

    with tile.TileContext(nc) as tc:
        with tc.tile_pool(name="const", bufs=1) as cpool:
            # early constants (phase A + L loop)
            wkvT_sb = cpool.tile([128, 4 * 1024], bf16)
            nc.sync.dma_start(wkvT_sb[:], wkvT[:, :])
            wkT_sb = cpool.tile([128, 16 * 128], bf16)
            nc.sync.dma_start(wkT_sb[:], wkT_in[:, :])
            sidx_sb = cpool.tile([128, W * 64], i16)
            nc.sync.dma_start(sidx_sb[:], sidx_in[:, :])
            drelL_sb = cpool.tile([128, W * BPW], f32)
            nc.sync.dma_start(drelL_sb[:], drelL_in[:, :])
            ident_sb = cpool.tile([128, 128], bf16)
            nc.sync.dma_start(ident_sb[:], ident_in[:, :])
            woT_sb = cpool.tile([128, 4 * 512], bf16)
            nc.sync.dma_start(woT_sb[:], woT[:, :])
            bo_sb = cpool.tile([128, 4], f32)
            nc.sync.dma_start(bo_sb[:], bo_in[:, :])
            iota_sb = cpool.tile([128, 128], bf16)
            nc.sync.dma_start(iota_sb[:], iota_in[:, :])
            ones_col = cpool.tile([128, 1], bf16)
            nc.vector.memset(ones_col[:], 1.0)
            ones_row = cpool.tile([1, 128], bf16)
            nc.vector.memset(ones_row[:], 1.0)
            # late constants (R loop only; loaded after phase A issues)
            vidx_sb = cpool.tile([128, W * 64], i16)
            eidx_sb = cpool.tile([128, W * 64], i16)
            drelR_sb = cpool.tile([128, W * BPW], f32)
            emask_sb = cpool.tile([128, W * BPW * 64], bf16)
            hacc = cpool.tile([128, 4 * npad], bf16)
            klT_sb = cpool.tile([128, 4, npad], bf16)   # left K transposed

            # ---- phase A ----
            with (
                tc.tile_pool(name="feat", bufs=1) as fpool,
                tc.tile_pool(name="gemm_sb", bufs=3) as gsb,
                tc.tile_pool(name="psum_gemm", bufs=2, space="PSUM") as pg,
            ):
                featR = []
                for cc in range(4):
                    t = fpool.tile([128, npad], bf16, tag=f"featR{cc}")
                    nc.sync.dma_start(
                        t[:], nT_in["R"][cc * 128:(cc + 1) * 128, :])
                    featR.append(t)
                # R side K|V fused GEMM -> tkv_shR -> AG#1
                for ti in range(ntiles):
                    sb = gsb.tile([128, 1024], bf16)
                    for half in range(2):
                        ps = pg.tile([128, 512], f32)
                        for cc in range(4):
                            nc.tensor.matmul(
                                ps[:],
                                lhsT=featR[cc][:, ti * 128:(ti + 1) * 128],
                                rhs=wkvT_sb[:, cc * 1024 + half * 512:
                                            cc * 1024 + half * 512 + 512],
                                start=(cc == 0), stop=(cc == 3))
                        if half == 0:
                            nc.vector.tensor_copy(
                                sb[:, 0:512], ps[:])
                        else:
                            nc.scalar.copy(
                                sb[:, 512:1024], ps[:])
                    nc.sync.dma_start(
                        tkv_shR[ti * 128:(ti + 1) * 128, :], sb[:])
                if not SKIP_AG:
                    nc.gpsimd.collective_compute(
                        "AllGather", mybir.AluOpType.bypass,
                        replica_groups=[list(range(NCORES))],
                        ins=[tkv_shR], outs=[tkv_R])
                else:
                    # timing build: sliver copy keeps the dependency edge
                    nc.sync.dma_start(tkv_R[0:128, :], tkv_shR[0:128, :])

                featL = []
                for cc in range(4):
                    t = fpool.tile([128, npad], bf16, tag=f"featL{cc}")
                    nc.sync.dma_start(
                        t[:], nT_in["L"][cc * 128:(cc + 1) * 128, :])
                    featL.append(t)
                # L side transposed-K GEMM -> klT_sb (stays in SBUF)
                for o in range(4):
                    for nb in range(math.ceil(npad / 512)):
                        n0 = nb * 512
                        n1 = min(npad, n0 + 512)
                        ps = pg.tile([128, 512], f32)
                        for i in range(4):
                            nc.tensor.matmul(
                                ps[:, 0:n1 - n0],
                                lhsT=wkT_sb[:, (i * 4 + o) * 128:
                                            (i * 4 + o + 1) * 128],
                                rhs=featL[i][:, n0:n1],
                                start=(i == 0), stop=(i == 3))
                        nc.scalar.copy(
                            klT_sb[:, o, n0:n1], ps[:, 0:n1 - n0])
                # L side V GEMM -> tv_shL -> AG#2
                for ti in range(ntiles):
                    sb = gsb.tile([128, 512], bf16, tag="sbv")
                    ps = pg.tile([128, 512], f32)
                    for cc in range(4):
                        nc.tensor.matmul(
                            ps[:],
                            lhsT=featL[cc][:, ti * 128:(ti + 1) * 128],
                            rhs=wkvT_sb[:, cc * 1024 + 512:
                                        cc * 1024 + 1024],
                            start=(cc == 0), stop=(cc == 3))
                    nc.vector.tensor_copy(sb[:], ps[:])
                    nc.sync.dma_start(
                        tv_shL[ti * 128:(ti + 1) * 128, :], sb[:])
                if not SKIP_AG:
                    nc.gpsimd.collective_compute(
                        "AllGather", mybir.AluOpType.bypass,
                        replica_groups=[list(range(NCORES))],
                        ins=[tv_shL], outs=[tv_L])
                else:
                    nc.sync.dma_start(tv_L[0:128, :], tv_shL[0:128, :])

            # late const loads (R loop)
            nc.sync.dma_start(vidx_sb[:], vidx_in[:, :])
            nc.sync.dma_start(eidx_sb[:], eidx_in[:, :])
            nc.sync.dma_start(drelR_sb[:], drelR_in[:, :])
            nc.sync.dma_start(emask_sb[:], emask_in[:, :])

            nidx_reg = nc.gpsimd.to_reg(WCAP)
            nidx_reg2 = nc.gpsimd.to_reg(WCAP // 2)

            with (
                tc.tile_pool(name="gath", bufs=3) as gpool,
                tc.tile_pool(name="aux2", bufs=3) as kdpool,
                tc.tile_pool(name="qtm", bufs=3) as qpool,
                tc.tile_pool(name="blk", bufs=4) as sp,
                tc.tile_pool(name="ebuf", bufs=4) as ebpool,
                tc.tile_pool(name="etbuf", bufs=2 * BPW) as ohpool,
                tc.tile_pool(name="tail", bufs=3) as tp,
                tc.tile_pool(name="pMT", bufs=2, space="PSUM") as pMT,
                tc.tile_pool(name="pmsg", bufs=2, space="PSUM") as pmsg,
                tc.tile_pool(name="paux", bufs=1, space="PSUM") as paux,
                tc.tile_pool(name="ph", bufs=1, space="PSUM") as ph,
            ):
                # ---- phase C-L: matmul scores + left messages ----
                for w in ([] if (SKIP_C or SKIP_L) else range(W)):
                    # transposed K gather split in two: the worker's ucode
                    # fails above 512 indices per transpose gather
                    ktgs = []
                    for h in range(2):
                        kt = gpool.tile([128, 4, WCAP // 2], bf16,
                                        tag=f"ktg{h}")
                        nc.gpsimd.dma_gather(
                            kt[:], tkv_R[:, 0:C],
                            sidx_sb[:, w * 64 + h * 32: w * 64 + h * 32 + 32],
                            WCAP // 2, nidx_reg2, C, elem_step=2 * C,
                            transpose=True)
                        ktgs.append(kt)
                    vg = gpool.tile([128, BPW, C], bf16, tag="vg")
                    nc.gpsimd.dma_gather(
                        vg[:], tkv_R[:, C:2 * C],
                        sidx_sb[:, w * 64:(w + 1) * 64],
                        WCAP, nidx_reg, C, elem_step=2 * C, queue_num=1)
                    qtm = qpool.tile([128, WCAP], bf16, tag="qtm")
                    nc.sync.dma_start(
                        qtm[:], qtm_in[:, w * WCAP:(w + 1) * WCAP])

                    ehs = sp.tile([128, BPW], f32, tag="ehs")
                    ets = []
                    msgT_ps = pmsg.tile([128, 512], f32)
                    z_ps = paux.tile([128, 128], f32, tag="aux")
                    for b in range(BPW):
                        M_ps = pMT.tile([128, 128], f32, tag="M")
                        kth = ktgs[b // 4]
                        bh = b % 4
                        for j in range(4):
                            nc.tensor.matmul(
                                M_ps[:],
                                lhsT=klT_sb[:, j, w * 128:(w + 1) * 128],
                                rhs=kth[:, j, bh * 128:(bh + 1) * 128],
                                start=(j == 0), stop=(j == 3))
                        eb = ebpool.tile([128, 128], bf16, tag="eb")
                        nc.scalar.activation(eb[:], M_ps[:], AF.Exp,
                                             scale=1.0 / TEMP)
                        em = ebpool.tile([128, 128], bf16, tag="em")
                        nc.vector.tensor_tensor(
                            em[:], eb[:],
                            qtm[:, b * 128:(b + 1) * 128], op=OP.mult)
                        T_ps = pMT.tile([128, 128], bf16, tag="T")
                        nc.tensor.transpose(T_ps[:], em[:], ident_sb[:])
                        et = ohpool.tile([128, 128], bf16, tag="et")
                        nc.scalar.copy(et[:], T_ps[:])
                        # eh per edge = row-sum of E^T (raw, pre-normalize)
                        nc.vector.tensor_reduce(
                            ehs[:, b:b + 1], et[:],
                            axis=mybir.AxisListType.X, op=OP.add)
                        ets.append(et)
                    # escore shard write: esc[w, p, b] = eh[p, b]
                    nc.sync.dma_start(esc_sh[w, :, :], ehs[:])

                    for cc in range(4):
                        for b in range(BPW):
                            nc.tensor.matmul(
                                msgT_ps[:, cc * 128:(cc + 1) * 128],
                                lhsT=vg[:, b, cc * 128:(cc + 1) * 128],
                                rhs=ets[b][:],
                                start=(b == 0), stop=(b == BPW - 1))
                    for b in range(BPW):
                        nc.tensor.matmul(
                            z_ps[0:1, :], lhsT=ones_col[:], rhs=ets[b][:],
                            start=(b == 0), stop=(b == BPW - 1))

                    _window_tail(nc, tc, mybir, w, msgT_ps, z_ps,
                                 tp, paux, ph, ones_row, woT_sb, bo_sb,
                                 hacc, W)
                for oc in ([] if (SKIP_C or SKIP_L) else range(4)):
                    nc.sync.dma_start(
                        hT_out["L"][oc * 128:(oc + 1) * 128, :],
                        hacc[:, oc * npad:(oc + 1) * npad])

                # ---- AG#3: escore ----
                if not SKIP_C:
                    if not SKIP_AG:
                        nc.gpsimd.collective_compute(
                            "AllGather", mybir.AluOpType.bypass,
                            replica_groups=[list(range(NCORES))],
                            ins=[esc_sh], outs=[esc_full])
                    else:
                        nc.sync.dma_start(esc_full[0:2, :],
                                          esc_sh[0, 0:16, :])

                # ---- phase C-R: right messages from shared scores ----
                for w in ([] if (SKIP_C or SKIP_R) else range(W)):
                    v = gpool.tile([128, BPW, C], bf16, tag="v")
                    nc.gpsimd.dma_gather(
                        v[:], tv_L[:, :], vidx_sb[:, w * 64:(w + 1) * 64],
                        WCAP, nidx_reg, C)
                    eg = kdpool.tile([128, BPW, 64], f32, tag="eg")
                    nc.gpsimd.dma_gather(
                        eg[:], esc_full[:, :], eidx_sb[:, w * 64:(w + 1) * 64],
                        WCAP, nidx_reg, 64, queue_num=1)

                    # select each edge's eh via the host-shipped one-hot mask
                    egm = sp.tile([128, BPW, 64], bf16, tag="egm")
                    nc.vector.tensor_tensor(
                        egm[:], eg[:, :, :],
                        emask_sb[:, w * BPW * 64:(w + 1) * BPW * 64],
                        op=OP.mult)
                    ehR = sp.tile([128, BPW], f32, tag="ehR")
                    nc.vector.tensor_reduce(
                        ehR[:], egm[:, :, :],
                        axis=mybir.AxisListType.X, op=OP.add)

                    ohs = []
                    for b in range(BPW):
                        oh = ohpool.tile([128, 128], bf16, tag="et")
                        nc.vector.tensor_scalar(
                            oh[:], iota_sb[:],
                            drelR_sb[:, w * BPW + b: w * BPW + b + 1],
                            ehR[:, b:b + 1], op0=OP.is_equal, op1=OP.mult)
                        ohs.append(oh)

                    msgT_ps = pmsg.tile([128, 512], f32)
                    z_ps = paux.tile([128, 128], f32, tag="aux")
                    for cc in range(4):
                        for b in range(BPW):
                            nc.tensor.matmul(
                                msgT_ps[:, cc * 128:(cc + 1) * 128],
                                lhsT=v[:, b, cc * 128:(cc + 1) * 128],
                                rhs=ohs[b][:],
                                start=(b == 0), stop=(b == BPW - 1))
                    for b in range(BPW):
                        nc.tensor.matmul(
                            z_ps[0:1, :], lhsT=ones_col[:], rhs=ohs[b][:],
                            start=(b == 0), stop=(b == BPW - 1))

                    _window_tail(nc, tc, mybir, w, msgT_ps, z_ps,
                                 tp, paux, ph, ones_row, woT_sb, bo_sb,
                                 hacc, W)
                for oc in ([] if (SKIP_C or SKIP_R) else range(4)):
                    nc.sync.dma_start(
                        hT_out["R"][oc * 128:(oc + 1) * 128, :],
                        hacc[:, oc * npad:(oc + 1) * npad])
    nc.compile()
    return nc


def _window_tail(nc, tc, mybir, w, msgT_ps, z_ps, tp, paux, ph,
                 ones_row, woT_sb, bo_sb, hacc, W):
    """z -> 1/z broadcast, msgT normalize, Wo GEMM, bias+LeakyReLU."""
    f32, bf16 = mybir.dt.float32, mybir.dt.bfloat16
    AF = mybir.ActivationFunctionType
    OP = mybir.AluOpType
    npad = W * 128

    zm = tp.tile([1, 128], f32, tag="zm")
    nc.vector.tensor_scalar_max(zm[:], z_ps[0:1, :], 1e-30)
    zr = tp.tile([1, 128], f32, tag="zr")
    nc.vector.reciprocal(zr[:], zm[:])
    zrb = tp.tile([1, 128], bf16, tag="zrb")
    nc.vector.tensor_copy(zrb[:], zr[:])
    zbc_ps = paux.tile([128, 128], f32, tag="aux")
    nc.tensor.matmul(zbc_ps[:], lhsT=ones_row[:], rhs=zrb[:],
                     start=True, stop=True)
    zbc = tp.tile([128, 128], f32, tag="zbc")
    nc.scalar.copy(zbc[:], zbc_ps[:])
    msgT_sb = tp.tile([128, 512], bf16, tag="msgT")
    for nch in range(4):
        nc.vector.tensor_tensor(
            msgT_sb[:, nch * 128:(nch + 1) * 128],
            msgT_ps[:, nch * 128:(nch + 1) * 128],
            zbc[:], op=OP.mult)
    hT_ps = ph.tile([128, 512], f32)
    for oc in range(4):
        for cc in range(4):
            nc.tensor.matmul(
                hT_ps[:, oc * 128:(oc + 1) * 128],
                lhsT=woT_sb[:, cc * 512 + oc * 128:
                            cc * 512 + oc * 128 + 128],
                rhs=msgT_sb[:, cc * 128:(cc + 1) * 128],
                start=(cc == 0), stop=(cc == 3))
    for oc in range(4):
        x = hacc[:, oc * npad + w * 128: oc * npad + (w + 1) * 128]
        nc.scalar.activation(
            x, hT_ps[:, oc * 128:(oc + 1) * 128],
            AF.Identity, bias=bo_sb[:, oc:oc + 1])
        x2 = tp.tile([128, 128], bf16, tag="x2")
        nc.vector.tensor_scalar_mul(x2[:], x, NEG)
        nc.vector.tensor_tensor(x, x, x2[:], op=OP.max)


def _assemble(results, arrs, key):
    out = np.zeros((N, C), np.float32)
    for c in range(NCORES):
        hT = np.asarray(results[c][key], np.float32)
        cn = arrs[c]["colnode"]
        m = cn >= 0
        out[cn[m]] = hT[:, m].T
    return out


_RUN_KWARGS = {}


def kernel(**inputs):
    from concourse.bass_utils import run_bass_kernel_spmd

    in_maps, arrL, arrR, W = _host_inputs(inputs)
    nc = _build_program(W)
    res = run_bass_kernel_spmd(nc, in_maps, core_ids=list(range(NCORES)),
                               **_RUN_KWARGS)
    out_l = _assemble(res.results, arrL, "hT_L")
    out_r = _assemble(res.results, arrR, "hT_R")
    kernel.last_results = res
    kernel.last_nc = nc
    kernel.last_W = W
    return (out_l, out_r)


# revision 31
# speedup vs baseline: 1.4522x; 1.0231x over previous
"""Trainium2 Bass kernel for nn_MultiHeadAttention_73589969649754
(gnn_message_passing / graph cross-attention).

v3 strategy (score sharing + matmul scoring):
  - Edges sorted by destination per side; core c owns nodes
    [c*2500, (c+1)*2500) on both sides.  Windows of <=128 consecutive
    nodes / <=1024 edge slots.  The HOST permutes each core's node
    columns so window w occupies slots [w*128, (w+1)*128) - all program
    offsets are SPMD-uniform and tables are slot-ordered.
  - Phase A: R side emits a fused K|V table (AllGather #1, 5.24MB/rank);
    L side computes K TRANSPOSED ([channel, slot], kept in SBUF - no
    DRAM round trip) and a V table (AllGather #2, 2.62MB/rank).
  - L pass per window: one TRANSPOSED dma_gather pulls Kr[src] in
    [channel, edge] orientation and a normal gather pulls Vr[src].
    Scores come from PE matmuls M[slot, edge] = KlT_win^T @ KrT_gath
    (no per-edge dot product on DVE/ACT), then exp on ACT, a host-shipped
    one-hot mask zeroes off-segment entries, and a PE transpose yields
    E^T[edge, slot] - which IS the eh-scaled one-hot the segment-sum
    matmuls consume.  eh per edge (row-sum of E^T) is written to a score
    shard (AllGather #3, 80KB/rank).  z via ones-matmul; messages are
    normalized after the fact by 1/z (column broadcast); Wo GEMM;
    bias+LeakyReLU.
  - R pass: gathers only Vl[src] rows (1KB/edge) plus 256B score-table
    rows; a host-shipped one-hot mask selects each edge's eh (softmax
    numerators are shared between the two sides).  No K gather, no dot
    product, no exp on the R side.
  - Outputs stored bf16 in [channel, slot] layout; host reassembles.
"""

import math

import numpy as np

N = 20000
E = 160000
C = 512
NCORES = 8
TEMP = float(np.sqrt(C))
NEG = 0.01
NPC = N // NCORES            # 2500 nodes per core per side
BLK = 128                    # edges per block
BPW = 8                      # blocks per window
WCAP = BPW * BLK             # 1024 edge slots per window
DUMMY_REL = 999.0
SKIP_AG = False
SKIP_C = False
SKIP_L = False       # debug: skip the L window loop
SKIP_R = False       # debug: skip the R window loop

# AllGather wall-time charge (ns): measured-table upper bound is ~70us for
# a 5.24MB/rank 8-way intra-chip AllGather; scale by bytes with a 20us
# latency floor.  Serial charges: AG#1 tkv_R 5.24MB -> 70us (gates the L
# loop) and AG#3 escore 80KB -> 20us (floor; gates the R score gathers).
# AG#2 (tv_L, 2.62MB -> ~35us) runs on the collective cores/links right
# after AG#1 (~105us done) and is only consumed by the R loop, which
# starts after the ~175us L loop - fully hidden, so not charged.
AG_CHARGE_NS = 70e3 + 20e3


def _prep_side(seg_dst):
    """Sort edges by dst; node-aligned core ranges; pack windows."""
    seg_dst = np.asarray(seg_dst, np.int64)
    perm = np.argsort(seg_dst, kind="stable")
    sd = seg_dst[perm]
    deg = np.bincount(sd, minlength=N)
    edge_b = [int(np.searchsorted(sd, c * NPC, "left"))
              for c in range(NCORES)] + [E]

    cores = []
    max_w = 0
    for c in range(NCORES):
        n0, n1 = c * NPC, (c + 1) * NPC
        e0 = edge_b[c]
        wins = []
        n, e = n0, e0
        while n < n1:
            wn = we = 0
            while n + wn < n1 and wn < BLK and we + deg[n + wn] <= WCAP:
                we += deg[n + wn]
                wn += 1
            assert wn > 0, "node degree exceeds window capacity"
            wins.append((n, wn, e, we))
            n += wn
            e += we
        assert e == edge_b[c + 1]
        cores.append((wins, e0))
        max_w = max(max_w, len(wins))
    return perm, sd, cores, max_w


def _wrap_idx16(idx_flat):
    """[n] -> [128, n//16] int16, i at [i%16, i//16], replicated x8."""
    n = idx_flat.shape[0]
    a = idx_flat.reshape(n // 16, 16).T.astype(np.int16)
    return np.ascontiguousarray(np.tile(a, (8, 1)))


def _slot_maps(cores, W):
    """Slot-order the nodes: window w of core c occupies slots
    [w*128, (w+1)*128).  Returns (slot_node [NCORES, W*128] node-or--1,
    node_slot [N] global slot = core*W*128 + slot)."""
    npad = W * BLK
    slot_node = np.full((NCORES, npad), -1, np.int64)
    node_slot = np.full(N, -1, np.int64)
    for c, (wins, e0) in enumerate(cores):
        for w, (fn, wn, es, ne) in enumerate(wins):
            sl = np.arange(wn)
            slot_node[c, w * BLK + sl] = fn + sl
            node_slot[fn + sl] = c * npad + w * BLK + sl
    assert (node_slot >= 0).all()
    return slot_node, node_slot


def _edge_locs(perm, cores):
    """Per original edge id: (core, window, slot-in-window) on this side."""
    core_of = np.empty(E, np.int32)
    win_of = np.empty(E, np.int32)
    slot_of = np.empty(E, np.int32)
    for c, (wins, e0) in enumerate(cores):
        for w, (fn, wn, es, ne) in enumerate(wins):
            ids = perm[es:es + ne]
            core_of[ids] = c
            win_of[ids] = w
            slot_of[ids] = np.arange(ne)
    return core_of, win_of, slot_of


def _host_inputs(inputs):
    import ml_dtypes
    bf16 = ml_dtypes.bfloat16

    nl = np.asarray(inputs["node_left"], np.float32)
    nr = np.asarray(inputs["node_right"], np.float32)
    Wk = np.asarray(inputs["Wk"], np.float32)
    Wv = np.asarray(inputs["Wv"], np.float32)
    Wo = np.asarray(inputs["Wo"], np.float32)
    bo = np.asarray(inputs["bo"], np.float32)
    sl = np.asarray(inputs["segmentation_index_left"], np.int64)
    sr = np.asarray(inputs["segmentation_index_right"], np.int64)

    permL, sdL, coresL, wL = _prep_side(sl)
    permR, sdR, coresR, wR = _prep_side(sr)
    W = max(wL, wR)
    npad = W * BLK

    snL, nsL = _slot_maps(coresL, W)     # left-node slots
    snR, nsR = _slot_maps(coresR, W)     # right-node slots
    LcoreE, LwinE, LslotE = _edge_locs(permL, coresL)

    # ---- per-core L arrays ----
    ssL = sr[permL]          # src (right) node per L-sorted edge
    arrL = []
    for c, (wins, e0) in enumerate(coresL):
        sidx = np.zeros((W, WCAP), np.int64)      # src rows in R table
        drel = np.full((W, WCAP), DUMMY_REL, np.float32)
        qtm = np.zeros((BLK, W * WCAP), np.float32)
        for w, (fn, wn, es, ne) in enumerate(wins):
            sidx[w, :ne] = nsR[ssL[es:es + ne]]
            rel = (sdL[es:es + ne] - fn)
            drel[w, :ne] = rel.astype(np.float32)
            # QT mask: [dst-rel, (w*8+b)*128 + p] = 1 for edge slot i=b*128+p
            i = np.arange(ne)
            qtm[rel, w * WCAP + (i // BLK) * BLK + (i % BLK)] = 1.0
        sidx16 = np.concatenate([_wrap_idx16(sidx[w]) for w in range(W)], 1)
        drelT = np.ascontiguousarray(
            drel.reshape(W, BPW, BLK).transpose(2, 0, 1).reshape(BLK, W * BPW))
        arrL.append(dict(sidx=sidx16, drel=drelT, qtm=qtm.astype(bf16),
                         colnode=snL[c]))

    # ---- per-core R arrays ----
    ssR = sl[permR]          # src (left) node per R-sorted edge
    arrR = []
    for c, (wins, e0) in enumerate(coresR):
        vidx = np.zeros((W, WCAP), np.int64)      # src rows in L V table
        eidx = np.zeros((W, WCAP), np.int64)      # escore row (64-col rows)
        ecol = np.zeros((W, WCAP), np.int64)
        emask_valid = np.zeros((W, WCAP), bool)
        drel = np.full((W, WCAP), DUMMY_REL, np.float32)
        for w, (fn, wn, es, ne) in enumerate(wins):
            ids = permR[es:es + ne]
            vidx[w, :ne] = nsL[ssR[es:es + ne]]
            # flat L escore position: core*(W*1024) + win*1024 + p*8 + b
            pL = LslotE[ids] % BLK
            bL = LslotE[ids] // BLK
            gpos = (LcoreE[ids].astype(np.int64) * W + LwinE[ids]) * WCAP \
                + pL * BPW + bL
            eidx[w, :ne] = gpos // 64
            ecol[w, :ne] = gpos % 64
            emask_valid[w, :ne] = True
            drel[w, :ne] = (sdR[es:es + ne] - fn).astype(np.float32)
        vidx16 = np.concatenate([_wrap_idx16(vidx[w]) for w in range(W)], 1)
        eidx16 = np.concatenate([_wrap_idx16(eidx[w]) for w in range(W)], 1)
        drelT = np.ascontiguousarray(
            drel.reshape(W, BPW, BLK).transpose(2, 0, 1).reshape(BLK, W * BPW))
        emask = np.zeros((BLK, W * BPW * 64), np.float32)
        wv, iv = np.nonzero(emask_valid)
        pv, bv = iv % BLK, iv // BLK
        emask[pv, (wv * BPW + bv) * 64 + ecol[wv, iv]] = 1.0
        arrR.append(dict(vidx=vidx16, eidx=eidx16, drel=drelT,
                         emask=emask.astype(bf16), colnode=snR[c]))

    # ---- shared constants ----
    Wkv = np.concatenate([Wk, Wv], 0)               # [1024, 512]
    WkvT = Wkv.T                                    # [512, 1024]
    wkvT_arr = np.zeros((128, 4 * 1024), np.float32)
    for cc in range(4):
        wkvT_arr[:, cc * 1024:(cc + 1) * 1024] = \
            WkvT[cc * 128:(cc + 1) * 128, :]
    # wkT: lhsT tiles for the transposed K GEMM
    wkT_arr = np.zeros((128, 16 * 128), np.float32)
    for i in range(4):
        for o in range(4):
            wkT_arr[:, (i * 4 + o) * 128:(i * 4 + o + 1) * 128] = \
                Wk[o * 128:(o + 1) * 128, i * 128:(i + 1) * 128].T
    woT_arr = np.zeros((128, 4 * 512), np.float32)
    for cc in range(4):
        for oc in range(4):
            woT_arr[:, cc * 512 + oc * 128: cc * 512 + (oc + 1) * 128] = \
                Wo[oc * 128:(oc + 1) * 128, cc * 128:(cc + 1) * 128].T
    bo_arr = bo.reshape(4, 128).T.copy()            # [128, 4]
    iota_arr = np.broadcast_to(
        np.arange(128, dtype=np.float32)[None, :], (128, 128))
    ident_arr = np.eye(128, dtype=np.float32)

    def shardT(feat, slot_node_c):
        sh = np.zeros((C, npad), np.float32)
        m = slot_node_c >= 0
        sh[:, m] = feat[slot_node_c[m]].T
        return np.ascontiguousarray(sh).astype(bf16)

    in_maps = []
    for c in range(NCORES):
        in_maps.append({
            "nT_L": shardT(nl, snL[c]),
            "nT_R": shardT(nr, snR[c]),
            "wkvT": wkvT_arr.astype(bf16),
            "wkT": wkT_arr.astype(bf16),
            "woT": woT_arr.astype(bf16),
            "bo": bo_arr,
            "iota": np.ascontiguousarray(iota_arr).astype(bf16),
            "ident": np.ascontiguousarray(ident_arr).astype(bf16),
            "sidx_L": arrL[c]["sidx"],
            "drel_L": arrL[c]["drel"],
            "qtm_L": arrL[c]["qtm"],
            "vidx_R": arrR[c]["vidx"],
            "eidx_R": arrR[c]["eidx"],
            "drel_R": arrR[c]["drel"],
            "emask_R": arrR[c]["emask"],
        })
    return in_maps, arrL, arrR, W


def _build_program(W):
    import concourse.bacc as bacc
    import concourse.tile as tile
    from concourse import mybir

    dt = mybir.dt
    f32, bf16, i16 = dt.float32, dt.bfloat16, dt.int16
    AF = mybir.ActivationFunctionType
    OP = mybir.AluOpType
    npad = W * BLK
    ntiles = W

    nc = bacc.Bacc("TRN2", target_bir_lowering=False, debug=False,
                   enable_asserts=True, num_devices=NCORES,
                   num_swdge_queues=2)

    # ---- I/O ----
    nT_in = {s: nc.dram_tensor(f"nT_{s}", [C, npad], bf16,
                               kind="ExternalInput").ap() for s in "LR"}
    wkvT = nc.dram_tensor("wkvT", [128, 4 * 1024], bf16,
                          kind="ExternalInput").ap()
    wkT_in = nc.dram_tensor("wkT", [128, 16 * 128], bf16,
                            kind="ExternalInput").ap()
    woT = nc.dram_tensor("woT", [128, 4 * 512], bf16,
                         kind="ExternalInput").ap()
    bo_in = nc.dram_tensor("bo", [128, 4], f32, kind="ExternalInput").ap()
    iota_in = nc.dram_tensor("iota", [128, 128], bf16,
                             kind="ExternalInput").ap()
    ident_in = nc.dram_tensor("ident", [128, 128], bf16,
                              kind="ExternalInput").ap()
    sidx_in = nc.dram_tensor("sidx_L", [128, W * 64], i16,
                             kind="ExternalInput").ap()
    drelL_in = nc.dram_tensor("drel_L", [128, W * BPW], f32,
                              kind="ExternalInput").ap()
    qtm_in = nc.dram_tensor("qtm_L", [128, W * WCAP], bf16,
                            kind="ExternalInput").ap()
    vidx_in = nc.dram_tensor("vidx_R", [128, W * 64], i16,
                             kind="ExternalInput").ap()
    eidx_in = nc.dram_tensor("eidx_R", [128, W * 64], i16,
                             kind="ExternalInput").ap()
    drelR_in = nc.dram_tensor("drel_R", [128, W * BPW], f32,
                              kind="ExternalInput").ap()
    emask_in = nc.dram_tensor("emask_R", [128, W * BPW * 64], bf16,
                              kind="ExternalInput").ap()
    hT_out = {s: nc.dram_tensor(f"hT_{s}", [C, npad], bf16,
                                kind="ExternalOutput").ap() for s in "LR"}

    # ---- internal DRAM ----
    tkv_shR = nc.dram_tensor("tkv_shR", [npad, 2 * C], bf16).ap()
    tv_shL = nc.dram_tensor("tv_shL", [npad, C], bf16).ap()
    esc_sh = nc.dram_tensor("esc_sh", [W, 128, BPW], f32).ap()
    tkv_R = nc.dram_tensor("tkv_R", [NCORES * npad, 2 * C], bf16,
                           addr_space="Shared").ap()
    tv_L = nc.dram_tensor("tv_L", [NCORES * npad, C], bf16,
                          addr_space="Shared").ap()
    esc_full = nc.dram_tensor("esc_full", [NCORES * W * 16, 64], f32,
                              addr_space="Shared").ap()

    with tile.TileContext(nc) as tc:
        with tc.tile_pool(name="const", bufs=1) as cpool:
            # early constants (phase A + L loop)
            wkvT_sb = cpool.tile([128, 4 * 1024], bf16)
            nc.sync.dma_start(wkvT_sb[:], wkvT[:, :])
            wkT_sb = cpool.tile([128, 16 * 128], bf16)
            nc.sync.dma_start(wkT_sb[:], wkT_in[:, :])
            sidx_sb = cpool.tile([128, W * 64], i16)
            nc.sync.dma_start(sidx_sb[:], sidx_in[:, :])
            drelL_sb = cpool.tile([128, W * BPW], f32)
            nc.sync.dma_start(drelL_sb[:], drelL_in[:, :])
            ident_sb = cpool.tile([128, 128], bf16)
            nc.sync.dma_start(ident_sb[:], ident_in[:, :])
            woT_sb = cpool.tile([128, 4 * 512], bf16)
            nc.sync.dma_start(woT_sb[:], woT[:, :])
            bo_sb = cpool.tile([128, 4], f32)
            nc.sync.dma_start(bo_sb[:], bo_in[:, :])
            iota_sb = cpool.tile([128, 128], bf16)
            nc.sync.dma_start(iota_sb[:], iota_in[:, :])
            ones_col = cpool.tile([128, 1], bf16)
            nc.vector.memset(ones_col[:], 1.0)
            ones_row = cpool.tile([1, 128], bf16)
            nc.vector.memset(ones_row[:], 1.0)
            # late constants (R loop only; loaded after phase A issues)
            vidx_sb = cpool.tile([128, W * 64], i16)
            eidx_sb = cpool.tile([128, W * 64], i16)
            drelR_sb = cpool.tile([128, W * BPW], f32)
            emask_sb = cpool.tile([128, W * BPW * 64], bf16)
            hacc = cpool.tile([128, 4 * npad], bf16)
            klT_sb = cpool.tile([128, 4, npad], bf16)   # left K transposed

            # ---- phase A ----
            with (
                tc.tile_pool(name="feat", bufs=1) as fpool,
                tc.tile_pool(name="gemm_sb", bufs=3) as gsb,
                tc.tile_pool(name="psum_gemm", bufs=2, space="PSUM") as pg,
            ):
                featR = []
                for cc in range(4):
                    t = fpool.tile([128, npad], bf16, tag=f"featR{cc}")
                    nc.sync.dma_start(
                        t[:], nT_in["R"][cc * 128:(cc + 1) * 128, :])
                    featR.append(t)
                # R side K|V fused GEMM -> tkv_shR -> AG#1
                for ti in range(ntiles):
                    sb = gsb.tile([128, 1024], bf16)
                    for half in range(2):
                        ps = pg.tile([128, 512], f32)
                        for cc in range(4):
                            nc.tensor.matmul(
                                ps[:],
                                lhsT=featR[cc][:, ti * 128:(ti + 1) * 128],
                                rhs=wkvT_sb[:, cc * 1024 + half * 512:
                                            cc * 1024 + half * 512 + 512],
                                start=(cc == 0), stop=(cc == 3))
                        if half == 0:
                            nc.vector.tensor_copy(
                                sb[:, 0:512], ps[:])
                        else:
                            nc.scalar.copy(
                                sb[:, 512:1024], ps[:])
                    nc.sync.dma_start(
                        tkv_shR[ti * 128:(ti + 1) * 128, :], sb[:])
                if not SKIP_AG:
                    nc.gpsimd.collective_compute(
                        "AllGather", mybir.AluOpType.bypass,
                        replica_groups=[list(range(NCORES))],
                        ins=[tkv_shR], outs=[tkv_R])
                else:
                    # timing build: sliver copy keeps the dependency edge
                    nc.sync.dma_start(tkv_R[0:128, :], tkv_shR[0:128, :])

                featL = []
                for cc in range(4):
                    t = fpool.tile([128, npad], bf16, tag=f"featL{cc}")
                    nc.sync.dma_start(
                        t[:], nT_in["L"][cc * 128:(cc + 1) * 128, :])
                    featL.append(t)
                # L side transposed-K GEMM -> klT_sb (stays in SBUF)
                for o in range(4):
                    for nb in range(math.ceil(npad / 512)):
                        n0 = nb * 512
                        n1 = min(npad, n0 + 512)
                        ps = pg.tile([128, 512], f32)
                        for i in range(4):
                            nc.tensor.matmul(
                                ps[:, 0:n1 - n0],
                                lhsT=wkT_sb[:, (i * 4 + o) * 128:
                                            (i * 4 + o + 1) * 128],
                                rhs=featL[i][:, n0:n1],
                                start=(i == 0), stop=(i == 3))
                        nc.scalar.copy(
                            klT_sb[:, o, n0:n1], ps[:, 0:n1 - n0])
                # L side V GEMM -> tv_shL -> AG#2
                for ti in range(ntiles):
                    sb = gsb.tile([128, 512], bf16, tag="sbv")
                    ps = pg.tile([128, 512], f32)
                    for cc in range(4):
                        nc.tensor.matmul(
                            ps[:],
                            lhsT=featL[cc][:, ti * 128:(ti + 1) * 128],
                            rhs=wkvT_sb[:, cc * 1024 + 512:
                                        cc * 1024 + 1024],
                            start=(cc == 0), stop=(cc == 3))
                    nc.vector.tensor_copy(sb[:], ps[:])
                    nc.sync.dma_start(
                        tv_shL[ti * 128:(ti + 1) * 128, :], sb[:])
                if not SKIP_AG:
                    nc.gpsimd.collective_compute(
                        "AllGather", mybir.AluOpType.bypass,
                        replica_groups=[list(range(NCORES))],
                        ins=[tv_shL], outs=[tv_L])
                else:
                    nc.sync.dma_start(tv_L[0:128, :], tv_shL[0:128, :])

            # late const loads (R loop)
            nc.sync.dma_start(vidx_sb[:], vidx_in[:, :])
            nc.sync.dma_start(eidx_sb[:], eidx_in[:, :])
            nc.sync.dma_start(drelR_sb[:], drelR_in[:, :])
            nc.sync.dma_start(emask_sb[:], emask_in[:, :])

            nidx_reg = nc.gpsimd.to_reg(WCAP)
            nidx_reg2 = nc.gpsimd.to_reg(WCAP // 2)

            with (
                tc.tile_pool(name="gath", bufs=3) as gpool,
                tc.tile_pool(name="aux2", bufs=3) as kdpool,
                tc.tile_pool(name="qtm", bufs=3) as qpool,
                tc.tile_pool(name="blk", bufs=4) as sp,
                tc.tile_pool(name="ebuf", bufs=4) as ebpool,
                tc.tile_pool(name="etbuf", bufs=2 * BPW) as ohpool,
                tc.tile_pool(name="tail", bufs=3) as tp,
                tc.tile_pool(name="pMT", bufs=2, space="PSUM") as pMT,
                tc.tile_pool(name="pmsg", bufs=2, space="PSUM") as pmsg,
                tc.tile_pool(name="paux", bufs=1, space="PSUM") as paux,
                tc.tile_pool(name="ph", bufs=1, space="PSUM") as ph,
            ):
                # ---- phase C-L: matmul scores + left messages ----
                for w in ([] if (SKIP_C or SKIP_L) else range(W)):
                    # transposed K gather split in two: the worker's ucode
                    # fails above 512 indices per transpose gather
                    ktgs = []
                    for h in range(2):
                        kt = gpool.tile([128, 4, WCAP // 2], bf16,
                                        tag=f"ktg{h}")
                        nc.gpsimd.dma_gather(
                            kt[:], tkv_R[:, 0:C],
                            sidx_sb[:, w * 64 + h * 32: w * 64 + h * 32 + 32],
                            WCAP // 2, nidx_reg2, C, elem_step=2 * C,
                            transpose=True)
                        ktgs.append(kt)
                    vg = gpool.tile([128, BPW, C], bf16, tag="vg")
                    nc.gpsimd.dma_gather(
                        vg[:], tkv_R[:, C:2 * C],
                        sidx_sb[:, w * 64:(w + 1) * 64],
                        WCAP, nidx_reg, C, elem_step=2 * C, queue_num=1)
                    qtm = qpool.tile([128, WCAP], bf16, tag="qtm")
                    nc.sync.dma_start(
                        qtm[:], qtm_in[:, w * WCAP:(w + 1) * WCAP])

                    ehs = sp.tile([128, BPW], f32, tag="ehs")
                    ets = []
                    msgT_ps = pmsg.tile([128, 512], f32)
                    z_ps = paux.tile([128, 128], f32, tag="aux")
                    for b in range(BPW):
                        M_ps = pMT.tile([128, 128], f32, tag="M")
                        kth = ktgs[b // 4]
                        bh = b % 4
                        for j in range(4):
                            nc.tensor.matmul(
                                M_ps[:],
                                lhsT=klT_sb[:, j, w * 128:(w + 1) * 128],
                                rhs=kth[:, j, bh * 128:(bh + 1) * 128],
                                start=(j == 0), stop=(j == 3))
                        eb = ebpool.tile([128, 128], bf16, tag="eb")
                        nc.scalar.activation(eb[:], M_ps[:], AF.Exp,
                                             scale=1.0 / TEMP)
                        em = ebpool.tile([128, 128], bf16, tag="em")
                        nc.vector.tensor_tensor(
                            em[:], eb[:],
                            qtm[:, b * 128:(b + 1) * 128], op=OP.mult)
                        T_ps = pMT.tile([128, 128], bf16, tag="T")
                        nc.tensor.transpose(T_ps[:], em[:], ident_sb[:])
                        et = ohpool.tile([128, 128], bf16, tag="et")
                        if b % 2 == 0:
                            nc.scalar.copy(et[:], T_ps[:])
                        else:
                            nc.vector.tensor_copy(et[:], T_ps[:])
                        # eh per edge = row-sum of E^T (raw, pre-normalize)
                        nc.vector.tensor_reduce(
                            ehs[:, b:b + 1], et[:],
                            axis=mybir.AxisListType.X, op=OP.add)
                        ets.append(et)
                    # escore shard write: esc[w, p, b] = eh[p, b]
                    nc.sync.dma_start(esc_sh[w, :, :], ehs[:])

                    for cc in range(4):
                        for b in range(BPW):
                            nc.tensor.matmul(
                                msgT_ps[:, cc * 128:(cc + 1) * 128],
                                lhsT=vg[:, b, cc * 128:(cc + 1) * 128],
                                rhs=ets[b][:],
                                start=(b == 0), stop=(b == BPW - 1))
                    for b in range(BPW):
                        nc.tensor.matmul(
                            z_ps[0:1, :], lhsT=ones_col[:], rhs=ets[b][:],
                            start=(b == 0), stop=(b == BPW - 1))

                    _window_tail(nc, tc, mybir, w, msgT_ps, z_ps,
                                 tp, paux, ph, ones_row, woT_sb, bo_sb,
                                 hacc, W)
                for oc in ([] if (SKIP_C or SKIP_L) else range(4)):
                    nc.sync.dma_start(
                        hT_out["L"][oc * 128:(oc + 1) * 128, :],
                        hacc[:, oc * npad:(oc + 1) * npad])

                # prefetch the first R v-gathers (need only AG#2) so
                # their transfers overlap the L-loop tail and AG#3
                vpre = []
                for w in ([] if (SKIP_C or SKIP_R) else range(2)):
                    v = gpool.tile([128, BPW, C], bf16, tag="v")
                    nc.gpsimd.dma_gather(
                        v[:], tv_L[:, :], vidx_sb[:, w * 64:(w + 1) * 64],
                        WCAP, nidx_reg, C)
                    vpre.append(v)

                # ---- AG#3: escore ----
                if not SKIP_C:
                    if not SKIP_AG:
                        nc.gpsimd.collective_compute(
                            "AllGather", mybir.AluOpType.bypass,
                            replica_groups=[list(range(NCORES))],
                            ins=[esc_sh], outs=[esc_full])
                    else:
                        nc.sync.dma_start(esc_full[0:2, :],
                                          esc_sh[0, 0:16, :])

                # ---- phase C-R: right messages from shared scores ----
                for w in ([] if (SKIP_C or SKIP_R) else range(W)):
                    if w < len(vpre):
                        v = vpre[w]
                    else:
                        v = gpool.tile([128, BPW, C], bf16, tag="v")
                        nc.gpsimd.dma_gather(
                            v[:], tv_L[:, :], vidx_sb[:, w * 64:(w + 1) * 64],
                            WCAP, nidx_reg, C)
                    eg = kdpool.tile([128, BPW, 64], f32, tag="eg")
                    nc.gpsimd.dma_gather(
                        eg[:], esc_full[:, :], eidx_sb[:, w * 64:(w + 1) * 64],
                        WCAP, nidx_reg, 64, queue_num=1)

                    # select each edge's eh via the host-shipped one-hot mask
                    egm = sp.tile([128, BPW, 64], bf16, tag="egm")
                    nc.vector.tensor_tensor(
                        egm[:], eg[:, :, :],
                        emask_sb[:, w * BPW * 64:(w + 1) * BPW * 64],
                        op=OP.mult)
                    ehR = sp.tile([128, BPW], f32, tag="ehR")
                    nc.vector.tensor_reduce(
                        ehR[:], egm[:, :, :],
                        axis=mybir.AxisListType.X, op=OP.add)

                    ohs = []
                    for b in range(BPW):
                        oh = ohpool.tile([128, 128], bf16, tag="et")
                        nc.vector.tensor_scalar(
                            oh[:], iota_sb[:],
                            drelR_sb[:, w * BPW + b: w * BPW + b + 1],
                            ehR[:, b:b + 1], op0=OP.is_equal, op1=OP.mult)
                        ohs.append(oh)

                    msgT_ps = pmsg.tile([128, 512], f32)
                    z_ps = paux.tile([128, 128], f32, tag="aux")
                    for cc in range(4):
                        for b in range(BPW):
                            nc.tensor.matmul(
                                msgT_ps[:, cc * 128:(cc + 1) * 128],
                                lhsT=v[:, b, cc * 128:(cc + 1) * 128],
                                rhs=ohs[b][:],
                                start=(b == 0), stop=(b == BPW - 1))
                    for b in range(BPW):
                        nc.tensor.matmul(
                            z_ps[0:1, :], lhsT=ones_col[:], rhs=ohs[b][:],
                            start=(b == 0), stop=(b == BPW - 1))

                    _window_tail(nc, tc, mybir, w, msgT_ps, z_ps,
                                 tp, paux, ph, ones_row, woT_sb, bo_sb,
                                 hacc, W)
                for oc in ([] if (SKIP_C or SKIP_R) else range(4)):
                    nc.sync.dma_start(
                        hT_out["R"][oc * 128:(oc + 1) * 128, :],
                        hacc[:, oc * npad:(oc + 1) * npad])
    nc.compile()
    return nc


def _window_tail(nc, tc, mybir, w, msgT_ps, z_ps, tp, paux, ph,
                 ones_row, woT_sb, bo_sb, hacc, W):
    """z -> 1/z broadcast, msgT normalize, Wo GEMM, bias+LeakyReLU."""
    f32, bf16 = mybir.dt.float32, mybir.dt.bfloat16
    AF = mybir.ActivationFunctionType
    OP = mybir.AluOpType
    npad = W * 128

    zm = tp.tile([1, 128], f32, tag="zm")
    nc.vector.tensor_scalar_max(zm[:], z_ps[0:1, :], 1e-30)
    zr = tp.tile([1, 128], f32, tag="zr")
    nc.vector.reciprocal(zr[:], zm[:])
    zrb = tp.tile([1, 128], bf16, tag="zrb")
    nc.vector.tensor_copy(zrb[:], zr[:])
    zbc_ps = paux.tile([128, 128], f32, tag="aux")
    nc.tensor.matmul(zbc_ps[:], lhsT=ones_row[:], rhs=zrb[:],
                     start=True, stop=True)
    zbc = tp.tile([128, 128], f32, tag="zbc")
    nc.scalar.copy(zbc[:], zbc_ps[:])
    msgT_sb = tp.tile([128, 512], bf16, tag="msgT")
    for nch in range(4):
        nc.vector.tensor_tensor(
            msgT_sb[:, nch * 128:(nch + 1) * 128],
            msgT_ps[:, nch * 128:(nch + 1) * 128],
            zbc[:], op=OP.mult)
    hT_ps = ph.tile([128, 512], f32)
    for oc in range(4):
        for cc in range(4):
            nc.tensor.matmul(
                hT_ps[:, oc * 128:(oc + 1) * 128],
                lhsT=woT_sb[:, cc * 512 + oc * 128:
                            cc * 512 + oc * 128 + 128],
                rhs=msgT_sb[:, cc * 128:(cc + 1) * 128],
                start=(cc == 0), stop=(cc == 3))
    for oc in range(4):
        x = hacc[:, oc * npad + w * 128: oc * npad + (w + 1) * 128]
        nc.scalar.activation(
            x, hT_ps[:, oc * 128:(oc + 1) * 128],
            AF.Identity, bias=bo_sb[:, oc:oc + 1])
        x2 = tp.tile([128, 128], bf16, tag="x2")
        nc.vector.tensor_scalar_mul(x2[:], x, NEG)
        nc.vector.tensor_tensor(x, x, x2[:], op=OP.max)


def _assemble(results, arrs, key):
    out = np.zeros((N, C), np.float32)
    for c in range(NCORES):
        hT = np.asarray(results[c][key], np.float32)
        cn = arrs[c]["colnode"]
        m = cn >= 0
        out[cn[m]] = hT[:, m].T
    return out


_RUN_KWARGS = {}


def kernel(**inputs):
    from concourse.bass_utils import run_bass_kernel_spmd

    in_maps, arrL, arrR, W = _host_inputs(inputs)
    nc = _build_program(W)
    res = run_bass_kernel_spmd(nc, in_maps, core_ids=list(range(NCORES)),
                               **_RUN_KWARGS)
    out_l = _assemble(res.results, arrL, "hT_L")
    out_r = _assemble(res.results, arrR, "hT_R")
    kernel.last_results = res
    kernel.last_nc = nc
    kernel.last_W = W
    return (out_l, out_r)
